# revision 1
# baseline (speedup 1.0000x reference)
"""GNN message-passing encoder (GAT-style) on 8 Trainium2 NeuronCores.

Self-contained: hardcodes the problem shapes (N=100000, E=1600000, HN=64, L=3).

Sharding: core c owns dst nodes [c*NL, (c+1)*NL). Host sorts each core's
edges by (src_chunk, dst), packs whole dst-segments into 128-edge tiles
(<=31 segments/tile + trash slot 31), 32 tiles/block; src chunks of 32768
keep dma_gather's int16 indices in range.

Per layer, on device:
- table build: hlT = fc_w[l]^T h; a_s = W_src hl; a_d = W_dst hl + biases;
  write row tables to HBM; AllGather node table [a_s|hl] (512B fp32 rows).
- per block: dma_gather 4096 src rows + 1024 a_d-window rows (bf16);
  PE expand matmul (M_T x window; row 31 of M_T = ea, row 31 of window = u_l,
  fusing the ea*u edge term into the same contraction);
  alpha = expand + a_s; ACT Lrelu + Exp (softmax without max subtraction --
  safe here since |alpha| stays O(10)); contrib = [ex*hl | ex] in bf16;
  PE aggregation matmul (M^T x contrib) -> per-segment partial sums;
  dma_scatter_add into per-src-chunk-group accumulators.
- finalize: h[n] = num/(den + 1e-16); feed next layer or write output.
"""
import os
import sys

for _p in ("/opt/trn_rl_repo",):
    if _p not in sys.path:
        sys.path.insert(0, _p)

import numpy as np
import ml_dtypes

import concourse.bass as bass
import concourse.mybir as mybir
import concourse.tile as tile
from concourse import bacc
from concourse import library_config as libcfg
from concourse.bass_utils import run_bass_kernel_spmd

F32 = mybir.dt.float32
BF16 = mybir.dt.bfloat16
I16 = mybir.dt.int16
AX = mybir.AluOpType
AF = mybir.ActivationFunctionType
BF16NP = ml_dtypes.bfloat16


class Cfg:
    def __init__(self, N=100000, E=1600000, C=8, HN=64, IN_N=3, L=3, B=10,
                 CHUNK=32768, BT=32, NEG=0.2, EPS=1e-5):
        self.N, self.E, self.C, self.HN, self.IN_N, self.L, self.B = \
            N, E, C, HN, IN_N, L, B
        self.CHUNK, self.BT = CHUNK, BT
        self.NEG, self.EPS = NEG, EPS
        self.NL = N // C
        self.G = (N + CHUNK - 1) // CHUNK
        self.TE = 128
        self.SS = 32
        self.EB = self.TE * BT
        self.TRASH = self.NL        # accum trash row
        self.UROW = self.NL         # a_d table u-row
        self.NLP = -(-(self.NL + 1) // 128) * 128   # padded accum rows


# ------------------------------------------------------------- host prep ----
def _pack_idx16(vals):
    v = np.asarray(vals, dtype=np.int16)
    assert v.size % 16 == 0
    return np.tile(v.reshape(-1, 16).T, (8, 1)).copy()


def prep_core(cfg, src, dst, ea, c):
    NL, G, CHUNK, TE, SS = cfg.NL, cfg.G, cfg.CHUNK, cfg.TE, cfg.SS
    m = (dst >= c * NL) & (dst < (c + 1) * NL)
    src_c, dstl_c, ea_c = src[m], dst[m] - c * NL, ea[m]
    g_c = src_c // CHUNK
    out = []
    for g in range(G):
        mg = g_c == g
        sg, dg, eg = src_c[mg], dstl_c[mg], ea_c[mg]
        order = np.argsort(dg, kind="stable")
        sg, dg, eg = sg[order], dg[order], eg[order]
        if dg.size:
            bnd = np.flatnonzero(np.diff(dg)) + 1
            starts = np.concatenate([[0], bnd])
            ends = np.concatenate([bnd, [dg.size]])
        else:
            starts = ends = np.array([], dtype=np.int64)
        tiles, cur_e, cur_s, ne = [], [], [], 0
        for s0, s1 in zip(starts, ends):
            cnt = int(s1 - s0)
            assert cnt <= TE, f"segment larger than a tile: {cnt}"
            if ne + cnt > TE or len(cur_s) >= SS - 2:
                tiles.append((cur_e, cur_s))
                cur_e, cur_s, ne = [], [], 0
            cur_e.append((int(s0), int(s1)))
            cur_s.append(int(dg[s0]))
            ne += cnt
        if cur_s:
            tiles.append((cur_e, cur_s))
        out.append((tiles, sg, eg))
    return out


def build_streams(cfg, groups, nbg):
    G, CHUNK, TE, SS, BT, EB = cfg.G, cfg.CHUNK, cfg.TE, cfg.SS, cfg.BT, cfg.EB
    NB = sum(nbg)
    gidx = np.zeros(NB * EB, np.int16)
    earow = np.zeros((NB, 4, EB), np.float32)
    segslot = np.full((128, NB * BT), SS - 1, BF16NP)
    rep = np.full((NB, 128, EB), 99, BF16NP)
    for tib in range(BT):
        rep[:, 32 * (tib % 4):32 * (tib % 4) + 32, tib * TE:(tib + 1) * TE] = SS - 1
    adidx = np.zeros(NB * BT * SS, np.int16)
    scidx = np.full(NB * BT * SS, cfg.TRASH, np.int16)

    b0 = 0
    for g in range(G):
        tiles, sg, eg = groups[g]
        for ti in range(nbg[g] * BT):
            blk = b0 + ti // BT
            tib = ti % BT
            tcol = blk * BT + tib
            base_e = blk * EB + tib * TE
            base_s = tcol * SS
            adidx[base_s + 0] = cfg.UROW
            if ti < len(tiles):
                cur_e, cur_s = tiles[ti]
                p = 0
                for si, (s0, s1) in enumerate(cur_e):
                    n = s1 - s0
                    sl = slice(base_e + p, base_e + p + n)
                    gidx[sl] = (sg[s0:s1] - g * CHUNK).astype(np.int16)
                    qq = tib % 4
                    earow[blk, qq, tib * TE + p: tib * TE + p + n] = eg[s0:s1]
                    segslot[p:p + n, tcol] = si + 1
                    rep[blk, 32 * qq:32 * qq + 32,
                        tib * TE + p: tib * TE + p + n] = si + 1
                    p += n
                for si, dl in enumerate(cur_s):
                    adidx[base_s + 1 + si] = dl
                    scidx[base_s + 1 + si] = dl
        b0 += nbg[g]

    return dict(gidx=_pack_idx16(gidx), adidx=_pack_idx16(adidx),
                scidx=_pack_idx16(scidx), segslot=segslot, earow=earow, rep=rep)


def host_prepare(cfg, inputs):
    ei = np.asarray(inputs["edge_index"])
    src, dst = ei[0].astype(np.int64), ei[1].astype(np.int64)
    ea = np.asarray(inputs["edge_attr"], np.float32)[:, 0]
    per_core = [prep_core(cfg, src, dst, ea, c) for c in range(cfg.C)]
    nbg = [max(1, -(-max(len(pc[g][0]) for pc in per_core) // cfg.BT))
           for g in range(cfg.G)]

    HN, L = cfg.HN, cfg.L
    fnw = np.asarray(inputs["fc_node_w"], np.float32)
    fnb = np.asarray(inputs["fc_node_b"], np.float32)
    few = np.asarray(inputs["fc_edge_w"], np.float32)
    feb = np.asarray(inputs["fc_edge_b"], np.float32)
    gam = np.asarray(inputs["bn_gamma"], np.float32)
    bet = np.asarray(inputs["bn_beta"], np.float32)
    fcw = np.asarray(inputs["fc_w"], np.float32)
    fcb = np.asarray(inputs["fc_b"], np.float32)
    aw = np.asarray(inputs["attn_w"], np.float32)
    ab = np.asarray(inputs["attn_b"], np.float32)
    wdst, wsrc, we = aw[:, :HN, :], aw[:, HN:2 * HN, :], aw[:, 2 * HN:, :]
    u = np.stack([few[0] @ we[l] for l in range(L)])
    adb = np.stack([ab[l] + feb @ we[l] for l in range(L)])
    urow = np.zeros((L, 128), np.float32)
    urow[:, :HN] = u
    x = np.asarray(inputs["x"], np.float32)

    shared = dict(
        fnw=fnw, fnb=fnb.reshape(-1, 1), gam=gam.reshape(-1, 1),
        bet=bet.reshape(-1, 1),
        fcw=np.ascontiguousarray(fcw.transpose(1, 0, 2)),     # [64, L, 64]
        wsrc=np.ascontiguousarray(wsrc.transpose(1, 0, 2)),
        wdst=np.ascontiguousarray(wdst.transpose(1, 0, 2)),
        fcb=np.ascontiguousarray(fcb.T), adb=np.ascontiguousarray(adb.T),
        urow=urow,
        iota_mod=(np.arange(128) % 32).astype(BF16NP).reshape(128, 1),
        iota_bt=np.tile(np.arange(32, dtype=BF16NP), (128, cfg.BT)),
        ident=np.eye(128, dtype=np.float32),
    )
    in_maps = []
    for c in range(cfg.C):
        st = build_streams(cfg, per_core[c], nbg)
        im = dict(shared)
        im.update(st)
        im["xT"] = np.ascontiguousarray(x[c * cfg.NL:(c + 1) * cfg.NL].T)
        in_maps.append(im)
    return in_maps, nbg


# --------------------------------------------------------------- builder ----
def split_sync_waits(nc, max_waits=1):
    for f in nc.m.functions:
        for bb in f.blocks:
            old = bb.instructions
            if not any(i.sync_info and i.sync_info.on_wait
                       and len(i.sync_info.on_wait) > max_waits for i in old):
                continue
            new = []
            for ins in old:
                si = ins.sync_info
                if si is not None and si.on_wait and len(si.on_wait) > max_waits:
                    waits = list(si.on_wait)
                    extra, keep = waits[:-max_waits], waits[-max_waits:]
                    for j, w in enumerate(extra):
                        nop = mybir.InstNoOp(name=f"{ins.name}-wc{j}", ins=[], outs=[])
                        nop.engine = ins.engine
                        nop.sync_info = mybir.SyncInfo(on_wait=[w], on_update=[])
                        new.append(nop)
                    si.on_wait = keep
                new.append(ins)
            bb.instructions = new
    return nc


def build_nc(cfg, nbg):
    NL, G, CHUNK, TE, SS, BT, EB = (cfg.NL, cfg.G, cfg.CHUNK, cfg.TE, cfg.SS,
                                    cfg.BT, cfg.EB)
    HN, L, C, N, NLP = cfg.HN, cfg.L, cfg.C, cfg.N, cfg.NLP
    NB = sum(nbg)
    SLB = BT * SS
    SUB = min(8, BT)
    RG = [list(range(C))]
    gob = []
    for g in range(G):
        gob += [g] * nbg[g]

    nc = bacc.Bacc(None, target_bir_lowering=False, num_swdge_queues=4)
    din = {}

    def ext(name, shape, dt=F32):
        din[name] = nc.dram_tensor(name, shape, dt, kind="ExternalInput")

    ext("xT", [cfg.IN_N, NL])
    ext("gidx", [128, NB * EB // 16], I16)
    ext("adidx", [128, NB * SLB // 16], I16)
    ext("scidx", [128, NB * SLB // 16], I16)
    ext("segslot", [128, NB * BT], BF16)
    ext("earow", [NB, 4, EB])
    ext("rep", [NB, 128, EB], BF16)
    ext("fnw", [cfg.IN_N, HN]); ext("fnb", [HN, 1])
    ext("gam", [HN, 1]); ext("bet", [HN, 1])
    ext("fcw", [HN, L, HN]); ext("fcb", [HN, L])
    ext("wsrc", [HN, L, HN]); ext("wdst", [HN, L, HN]); ext("adb", [HN, L])
    ext("urow", [L, 128])
    ext("iota_mod", [128, 1], BF16); ext("iota_bt", [128, BT * SS], BF16)
    ext("ident", [128, 128])

    out = nc.dram_tensor("out", [NL, HN], F32, kind="ExternalOutput")
    local_table = nc.dram_tensor("local_table", [NL, 2 * HN], F32)
    full_table = nc.dram_tensor("full_table", [N, 2 * HN], F32,
                                addr_space="Shared")
    local_ad = nc.dram_tensor("local_ad", [NL + 1, 128], F32)
    accum = nc.dram_tensor("accum", [2 * G, NLP, 2 * HN], F32)
    bn_loc = nc.dram_tensor("bn_loc", [HN, 2], F32)
    bn_sh = nc.dram_tensor("bn_sh", [HN, 2], F32, addr_space="Shared")

    with tile.TileContext(nc) as tc:
        with tc.tile_pool(name="persist", bufs=1) as pp:
            nc.gpsimd.load_library(libcfg.mlp)
            hT = pp.tile([HN, NL], F32)
            w = {}
            for nm, shp, dt in (
                ("fnw", [cfg.IN_N, HN], F32), ("fnb", [HN, 1], F32),
                ("gam", [HN, 1], F32), ("bet", [HN, 1], F32),
                ("fcw", [HN, L, HN], F32), ("fcb", [HN, L], F32),
                ("wsrc", [HN, L, HN], F32), ("wdst", [HN, L, HN], F32),
                ("adb", [HN, L], F32),
                ("iota_mod", [128, 1], BF16), ("iota_bt", [128, BT * SS], BF16),
                ("ident", [128, 128], F32),
                ("segslot", [128, NB * BT], BF16),
            ):
                w[nm] = pp.tile(shp, dt, tag=nm, name=nm)
                nc.sync.dma_start(out=w[nm][:], in_=din[nm][:])
            zero2k = pp.tile([128, 2048], F32)
            nc.vector.memset(zero2k[:], 0.0)
            ident = w["ident"]

            # ---------------- input + BN ----------------
            with tc.tile_pool(name="bn", bufs=2) as bp, \
                 tc.tile_pool(name="bnsq", bufs=1) as bq, \
                 tc.tile_pool(name="bnp", bufs=2, space="PSUM") as bpp:
                xt = bq.tile([cfg.IN_N, NL], F32, tag="xt")
                nc.sync.dma_start(out=xt[:], in_=din["xT"][:])
                for ci in range(-(-NL // 512)):
                    c0 = ci * 512
                    n = min(512, NL - c0)
                    ps = bpp.tile([HN, 512], F32, tag="ps")
                    nc.tensor.matmul(out=ps[:, :n], lhsT=w["fnw"][:],
                                     rhs=xt[:, c0:c0 + n], start=True, stop=True)
                    nc.scalar.copy(out=hT[:, c0:c0 + n], in_=ps[:, :n])
                nc.vector.tensor_scalar_add(hT[:], hT[:], w["fnb"][:])
                st = bp.tile([HN, 2], F32, tag="st")
                sq = bq.tile([HN, NL], F32, tag="sq")
                nc.vector.reduce_sum(st[:, 0:1], hT[:], axis=mybir.AxisListType.X)
                nc.vector.scalar_tensor_tensor(out=sq[:], in0=hT[:], scalar=1.0,
                                               in1=hT[:], op0=AX.mult, op1=AX.mult,
                                               accum_out=st[:, 1:2])
                nc.sync.dma_start(out=bn_loc[:], in_=st[:])
                nc.gpsimd.collective_compute("AllReduce", AX.add,
                                             replica_groups=RG,
                                             ins=[bn_loc[:]], outs=[bn_sh[:]])
                sg = bp.tile([HN, 2], F32, tag="sg")
                nc.sync.dma_start(out=sg[:], in_=bn_sh[:])
                mean = bp.tile([HN, 1], F32, tag="mean")
                var = bp.tile([HN, 1], F32, tag="var")
                nc.vector.tensor_scalar_mul(mean[:], sg[:, 0:1], 1.0 / N)
                nc.vector.tensor_scalar_mul(var[:], sg[:, 1:2], 1.0 / N)
                msq = bp.tile([HN, 1], F32, tag="msq")
                nc.vector.tensor_mul(msq[:], mean[:], mean[:])
                nc.vector.tensor_sub(var[:], var[:], msq[:])
                nc.vector.tensor_scalar_add(var[:], var[:], cfg.EPS)
                rs = bp.tile([HN, 1], F32, tag="rs")
                nc.scalar.activation(out=rs[:], in_=var[:], func=AF.Sqrt)
                nc.vector.reciprocal(rs[:], rs[:])
                scale = bp.tile([HN, 1], F32, tag="scale")
                nc.vector.tensor_mul(scale[:], rs[:], w["gam"][:])
                nbias = bp.tile([HN, 1], F32, tag="nbias")
                nc.vector.tensor_mul(nbias[:], mean[:], scale[:])
                nc.vector.scalar_tensor_tensor(out=nbias[:], in0=nbias[:],
                                               scalar=-1.0, in1=w["bet"][:],
                                               op0=AX.mult, op1=AX.add)
                nc.vector.tensor_scalar(out=hT[:], in0=hT[:], scalar1=scale[:],
                                        scalar2=nbias[:], op0=AX.mult, op1=AX.add)

            # ---------------- layers ----------------
            for l in range(L):
                with tc.tile_pool(name=f"tb{l}", bufs=2) as tp, \
                     tc.tile_pool(name=f"tp{l}", bufs=2, space="PSUM") as tpp:
                    for ci in range(-(-NL // 512)):
                        c0 = ci * 512
                        n = min(512, NL - c0)
                        psh = tpp.tile([HN, 512], F32, tag="psh")
                        nc.tensor.matmul(out=psh[:, :n], lhsT=w["fcw"][:, l, :],
                                         rhs=hT[:, c0:c0 + n], start=True, stop=True)
                        hl = tp.tile([HN, 512], F32, tag="hl")
                        nc.scalar.copy(out=hl[:, :n], in_=psh[:, :n])
                        nc.vector.tensor_scalar_add(hl[:, :n], hl[:, :n],
                                                    w["fcb"][:, l:l + 1])
                        pss = tpp.tile([HN, 512], F32, tag="psh")
                        nc.tensor.matmul(out=pss[:, :n], lhsT=w["wsrc"][:, l, :],
                                         rhs=hl[:, :n], start=True, stop=True)
                        asb = tp.tile([HN, 512], F32, tag="asb")
                        nc.scalar.copy(out=asb[:, :n], in_=pss[:, :n])
                        psd = tpp.tile([HN, 512], F32, tag="psh")
                        nc.tensor.matmul(out=psd[:, :n], lhsT=w["wdst"][:, l, :],
                                         rhs=hl[:, :n], start=True, stop=True)
                        adt = tp.tile([HN, 512], F32, tag="adt")
                        nc.vector.tensor_scalar_add(adt[:, :n], psd[:, :n],
                                                    w["adb"][:, l:l + 1])
                        for si in range(-(-n // 128)):
                            s0, sn = si * 128, min(128, n - si * 128)
                            rows = tp.tile([128, 2 * HN], F32, tag="rows")
                            pt = tpp.tile([128, HN], F32, tag="pt")
                            nc.tensor.transpose(out=pt[:sn, :],
                                                in_=asb[:, s0:s0 + sn],
                                                identity=ident[:HN, :HN])
                            nc.scalar.copy(out=rows[:sn, 0:HN], in_=pt[:sn, :])
                            pt2 = tpp.tile([128, HN], F32, tag="pt")
                            nc.tensor.transpose(out=pt2[:sn, :],
                                                in_=hl[:, s0:s0 + sn],
                                                identity=ident[:HN, :HN])
                            nc.scalar.copy(out=rows[:sn, HN:2 * HN], in_=pt2[:sn, :])
                            nc.sync.dma_start(
                                out=local_table[c0 + s0:c0 + s0 + sn, :],
                                in_=rows[:sn, :])
                            adr = tp.tile([128, 128], F32, tag="adr")
                            nc.vector.memset(adr[:], 0.0)
                            pt3 = tpp.tile([128, HN], F32, tag="pt")
                            nc.tensor.transpose(out=pt3[:sn, :],
                                                in_=adt[:, s0:s0 + sn],
                                                identity=ident[:HN, :HN])
                            nc.vector.tensor_copy(adr[:sn, 0:HN], pt3[:sn, :])
                            nc.sync.dma_start(
                                out=local_ad[c0 + s0:c0 + s0 + sn, :],
                                in_=adr[:sn, :])
                    nc.sync.dma_start(out=local_ad[NL:NL + 1, :],
                                      in_=din["urow"][l:l + 1, :])
                    # zero accums
                    for g in range(2 * G):
                        a_tot = NLP // 128
                        off = 0
                        while off < a_tot:
                            an = min(14, a_tot - off)
                            nc.sync.dma_start(
                                out=accum[g].rearrange("(a p) f -> p a f", p=128)
                                [:, off:off + an, :],
                                in_=zero2k[:, :an * 128].rearrange(
                                    "p (a f) -> p a f", f=2 * HN))
                            off += an

                nc.gpsimd.collective_compute("AllGather", AX.bypass,
                                             replica_groups=RG,
                                             ins=[local_table[:]],
                                             outs=[full_table[:]])

                with tc.tile_pool(name=f"ep{l}", bufs=2) as ep, \
                     tc.tile_pool(name=f"pp{l}", bufs=2, space="PSUM") as epp:
                    for b in range(NB):
                        g = gob[b]
                        gix = ep.tile([128, EB // 16], I16, tag="gix")
                        nc.sync.dma_start(
                            out=gix[:],
                            in_=din["gidx"][:, b * (EB // 16):(b + 1) * (EB // 16)])
                        aix = ep.tile([128, SLB // 16], I16, tag="aix")
                        nc.sync.dma_start(
                            out=aix[:],
                            in_=din["adidx"][:, b * (SLB // 16):(b + 1) * (SLB // 16)])
                        six = ep.tile([128, SLB // 16], I16, tag="six")
                        nc.sync.dma_start(
                            out=six[:],
                            in_=din["scidx"][:, b * (SLB // 16):(b + 1) * (SLB // 16)])
                        srcr = ep.tile([128, BT, 2 * HN], F32, tag="srcr")
                        for hh in range(2):
                            nc.gpsimd.dma_gather(
                                out_ap=srcr[:, hh * (BT // 2):(hh + 1) * (BT // 2), :],
                                in_ap=full_table[g * CHUNK:min((g + 1) * CHUNK, N), :],
                                idxs_ap=gix[:, hh * (EB // 32):(hh + 1) * (EB // 32)],
                                num_idxs=EB // 2, num_idxs_reg=EB // 2,
                                elem_size=2 * HN,
                                single_packet=False, queue_num=(b + 2 * hh) % 4)
                        adw = ep.tile([128, SLB // 128, 128], F32, tag="adw")
                        nc.gpsimd.dma_gather(
                            out_ap=adw[:], in_ap=local_ad[:],
                            idxs_ap=aix[:],
                            num_idxs=SLB, num_idxs_reg=SLB, elem_size=128,
                            single_packet=False, queue_num=(b + 1) % 4)
                        rp = ep.tile([128, EB], BF16, tag="rp")
                        nc.sync.dma_start(out=rp[:], in_=din["rep"][b, :, :])
                        mt = ep.tile([128, EB], F32, tag="mt", bufs=1)
                        nc.vector.tensor_tensor(
                            out=mt[:], in0=w["iota_mod"][:].to_broadcast([128, EB]),
                            in1=rp[:], op=AX.is_equal)
                        for qq in range(4):
                            nc.sync.dma_start(
                                out=mt[32 * qq:32 * qq + 1, :],
                                in_=din["earow"][b, qq:qq + 1, :])

                        mm = ep.tile([128, BT, SS], F32, tag="mm", bufs=1)
                        ssl = w["segslot"][:, b * BT:(b + 1) * BT]
                        nc.vector.tensor_tensor(
                            out=mm[:],
                            in0=ssl.to_broadcast([128, BT, SS]),
                            in1=w["iota_bt"][:].rearrange(
                                "p (t s) -> p t s", s=SS),
                            op=AX.is_equal)
                        alpha = ep.tile([128, BT, HN], F32, tag="alpha")
                        contrib = ep.tile([128, BT, 2 * HN], F32, tag="contrib")
                        scat = ep.tile([128, SLB // 128, 2 * HN], F32, tag="scat")
                        for ww in range(BT // SUB):
                            psx = epp.tile([128, SUB * HN], F32, tag="psx")
                            for t2 in range(SUB):
                                t = ww * SUB + t2
                                po = (SS * t) % 128
                                nc.tensor.matmul(
                                    out=psx[:, t2 * HN:(t2 + 1) * HN],
                                    lhsT=mt[:, t * TE:(t + 1) * TE],
                                    rhs=adw[:, t // 4, 0:HN],
                                    start=True, stop=True)
                            nc.vector.tensor_tensor(
                                out=alpha[:, ww * SUB:(ww + 1) * SUB, :],
                                in0=psx[:].rearrange("p (t h) -> p t h", h=HN),
                                in1=srcr[:, ww * SUB:(ww + 1) * SUB, 0:HN],
                                op=AX.add)
                        nc.vector.scalar_tensor_tensor(
                            out=alpha[:], in0=alpha[:], scalar=cfg.NEG,
                            in1=alpha[:], op0=AX.mult, op1=AX.max)
                        nc.scalar.activation(out=contrib[:, :, HN:2 * HN],
                                             in_=alpha[:], func=AF.Exp)
                        nc.vector.tensor_mul(contrib[:, :, 0:HN],
                                             contrib[:, :, HN:2 * HN],
                                             srcr[:, :, HN:2 * HN])
                        for q in range(BT // 2):
                            psa = epp.tile([64, 2 * HN], F32, tag="psa")
                            for t2 in range(2):
                                t = q * 2 + t2
                                nc.tensor.matmul(
                                    out=psa[t2 * SS:(t2 + 1) * SS, :],
                                    lhsT=mm[:, t, :], rhs=contrib[:, t, :],
                                    start=True, stop=True)
                            nc.scalar.copy(
                                out=scat[64 * (q % 2):64 * (q % 2) + 64,
                                         q // 2, :],
                                in_=psa[:])
                        nc.gpsimd.dma_scatter_add(
                            accum[2 * g + (b % 2), :NL + 1, :], scat[:], six[:],
                            SLB, SLB, 2 * HN,
                            single_packet=False, queue_num=(b + 2) % 4)

                with tc.tile_pool(name=f"fi{l}", bufs=3) as fp, \
                     tc.tile_pool(name=f"fp{l}", bufs=3, space="PSUM") as fpp:
                    for ci in range(-(-NL // 128)):
                        r0 = ci * 128
                        n = min(128, NL - r0)
                        a4 = fp.tile([128, 2 * G, 2 * HN], F32, tag="a4")
                        nc.sync.dma_start(
                            out=a4[:n, :, :],
                            in_=accum[:, r0:r0 + n, :].rearrange("g n d -> n g d"))
                        s01 = fp.tile([128, 2 * HN], F32, tag="s01")
                        nc.vector.tensor_add(s01[:n, :], a4[:n, 0, :], a4[:n, 1, :])
                        for gg in range(2, 2 * G):
                            nc.vector.tensor_add(s01[:n, :], s01[:n, :],
                                                 a4[:n, gg, :])
                        pt = fpp.tile([128, 128], F32, tag="pt")
                        nc.tensor.transpose(out=pt[:, :n], in_=s01[:n, :],
                                            identity=ident[:n, :n])
                        den = fp.tile([HN, 128], F32, tag="den")
                        nc.vector.tensor_scalar_add(den[:, :n],
                                                    pt[HN:2 * HN, :n], 1e-16)
                        nc.vector.reciprocal(den[:, :n], den[:, :n])
                        if l < L - 1:
                            nc.vector.tensor_mul(hT[:, r0:r0 + n], pt[0:HN, :n],
                                                 den[:, :n])
                        else:
                            hout = fp.tile([HN, 128], F32, tag="hout")
                            nc.vector.tensor_mul(hout[:, :n], pt[0:HN, :n],
                                                 den[:, :n])
                            po = fpp.tile([128, HN], F32, tag="po")
                            nc.tensor.transpose(out=po[:n, :], in_=hout[:, :n],
                                                identity=ident[:HN, :HN])
                            orow = fp.tile([128, HN], F32, tag="orow")
                            nc.scalar.copy(out=orow[:n, :], in_=po[:n, :])
                            nc.sync.dma_start(out=out[r0:r0 + n, :],
                                              in_=orow[:n, :])
    return nc


# --------------------------------------------------------------- entry ------
def kernel(**inputs):
    cfg = Cfg()
    in_maps, nbg = host_prepare(cfg, inputs)
    nc = build_nc(cfg, nbg)
    nc.compile()
    split_sync_waits(nc)
    res = run_bass_kernel_spmd(nc, in_maps, core_ids=list(range(cfg.C)),
                               trace=bool(int(os.environ.get("GNN_TRACE", "0"))))
    if res.exec_time_ns is not None:
        print(f"HW exec time: {res.exec_time_ns} ns")
    h = np.concatenate([np.asarray(res.results[c]["out"])
                        for c in range(cfg.C)], axis=0)
    return h.reshape(cfg.B, -1, cfg.HN)



# revision 4
# speedup vs baseline: 1.4896x; 1.4896x over previous
"""GNN message-passing encoder (GAT-style) on 8 Trainium2 NeuronCores.

Self-contained: hardcodes the problem shapes (N=100000, E=1600000, HN=64, L=3).

Sharding: core c owns dst nodes [c*NL, (c+1)*NL). Edges sorted by
(src_chunk g, dst); dst space split into 128-aligned windows; per (g, window)
the tile count T_gw is the max over cores (shared compile-time schedule).

Per layer, on device (all bf16 tables):
- table build: hlT = fc_w[l]^T h; a_s = W_src hl; a_d = W_dst hl + bias;
  node rows [a_s|hl] -> local_table -> AllGather full_table (256B rows);
  a_d rows transposed into SBUF-resident ad_sb[128, NW, 64].
- per block (4096 edges): dma_gather 4096 src rows (bf16, 1 desc each);
  host-built one-hot mtw [128 dstoff x 4096 e] from HBM; expand matmul
  psx = mtw^T @ ad_sb[:, w, :] per tile -> per-edge a_d; alpha = psx + a_s
  + ea*u (broadcast DVE); LRelu; Exp -> contrib [ex*hl | ex] bf16;
  aggregation matmul per tile accumulates into per-window PSUM
  (mt_ew one-hot built on-device); window flush adds into SBUF acc (f32).
- finalize from SBUF acc: h = num/(den+1e-16) -> hT bf16 (or out rows f32).
No HBM scatter/accumulators; only per-edge gather remains indexed.
"""
import os
import sys

for _p in ("/opt/trn_rl_repo",):
    if _p not in sys.path:
        sys.path.insert(0, _p)

import numpy as np
import ml_dtypes

import concourse.bass as bass
import concourse.mybir as mybir
import concourse.tile as tile
from concourse import bacc
from concourse import library_config as libcfg
from concourse.bass_utils import run_bass_kernel_spmd

F32 = mybir.dt.float32
BF16 = mybir.dt.bfloat16
I16 = mybir.dt.int16
AX = mybir.AluOpType
AF = mybir.ActivationFunctionType
BF16NP = ml_dtypes.bfloat16


class Cfg:
    def __init__(self):
        self.N, self.E, self.C = 100000, 1600000, 8
        self.HN, self.IN_N, self.L, self.B = 64, 3, 3, 10
        self.NEG, self.EPS = 0.2, 1e-5
        self.NL = self.N // self.C          # 12500
        self.CHUNK = 25000
        self.G = self.N // self.CHUNK       # 4
        self.TE = 128
        self.BT = 32
        self.EB = self.TE * self.BT         # 4096
        self.NW = -(-self.NL // 128)        # 98
        self.NLP = self.NW * 128            # 12544


# ------------------------------------------------------------- host prep ----
def _pack_idx16(vals):
    v = np.asarray(vals, dtype=np.int16)
    assert v.size % 16 == 0
    return np.tile(v.reshape(-1, 16).T, (8, 1)).copy()


def host_prepare(cfg, inputs):
    C, G, NL, CHUNK, NW = cfg.C, cfg.G, cfg.NL, cfg.CHUNK, cfg.NW
    TE, BT, EB, HN, L = cfg.TE, cfg.BT, cfg.EB, cfg.HN, cfg.L

    ei = np.asarray(inputs["edge_index"]).astype(np.int64)
    src, dst = ei[0], ei[1]
    ea_all = np.asarray(inputs["edge_attr"], np.float32)[:, 0]
    core = dst // NL
    dstl = dst - core * NL
    gch = src // CHUNK
    win = dstl >> 7
    dof = dstl & 127

    # shared tile schedule: T_gw = max over cores of ceil(cnt/128)
    key = (core * G + gch) * NW + win
    cnt = np.bincount(key, minlength=C * G * NW).reshape(C, G, NW)
    Tgw = -(-cnt.max(axis=0) // TE)                    # [G, NW]
    tiles_g = Tgw.sum(axis=1)                          # [G]
    nbg = np.maximum(1, -(-tiles_g // BT))
    NB = int(nbg.sum())
    tile_start = np.cumsum(Tgw, axis=1) - Tgw          # [G, NW]
    block_base = np.concatenate([[0], np.cumsum(nbg)])[:G]

    # per-block window schedule (window id per tile slot, -1 = padding)
    wsched = -np.ones((NB, BT), np.int64)
    for g in range(G):
        flat = np.repeat(np.arange(NW), Tgw[g])
        b = block_base[g] + np.arange(len(flat)) // BT
        wsched[b, np.arange(len(flat)) % BT] = flat
    gob = np.repeat(np.arange(G), nbg)

    # weights
    fnw = np.asarray(inputs["fc_node_w"], np.float32)
    fnb = np.asarray(inputs["fc_node_b"], np.float32)
    few = np.asarray(inputs["fc_edge_w"], np.float32)
    feb = np.asarray(inputs["fc_edge_b"], np.float32)
    gam = np.asarray(inputs["bn_gamma"], np.float32)
    bet = np.asarray(inputs["bn_beta"], np.float32)
    fcw = np.asarray(inputs["fc_w"], np.float32)
    fcb = np.asarray(inputs["fc_b"], np.float32)
    aw = np.asarray(inputs["attn_w"], np.float32)
    ab = np.asarray(inputs["attn_b"], np.float32)
    wdst, wsrc, we = aw[:, :HN, :], aw[:, HN:2 * HN, :], aw[:, 2 * HN:, :]
    u = np.stack([few[0] @ we[l] for l in range(L)])          # [L, 64]
    adb = np.stack([ab[l] + feb @ we[l] for l in range(L)])   # [L, 64]
    # u replicated across partitions and tiled BT times along free dim
    u_tiled = np.concatenate(
        [np.tile(u[l].astype(BF16NP)[None, :], (128, BT)) for l in range(L)],
        axis=1)                                               # [128, L*BT*64]
    x = np.asarray(inputs["x"], np.float32)

    shared = dict(
        fnw=fnw, fnb=fnb.reshape(-1, 1), gam=gam.reshape(-1, 1),
        bet=bet.reshape(-1, 1),
        fcw=np.ascontiguousarray(fcw.transpose(1, 0, 2)).astype(BF16NP),
        wsrc=np.ascontiguousarray(wsrc.transpose(1, 0, 2)).astype(BF16NP),
        wdst=np.ascontiguousarray(wdst.transpose(1, 0, 2)).astype(BF16NP),
        fcb=np.ascontiguousarray(fcb.T), adb=np.ascontiguousarray(adb.T),
        u_tiled=u_tiled,
        iota_mod=np.tile(np.arange(128, dtype=BF16NP), (128, BT)),
        identb=np.eye(128, dtype=BF16NP),
        identf=np.eye(128, dtype=np.float32),
    )

    in_maps = []
    for c in range(C):
        m = core == c
        sc, gc, wc, dfc, eac, dlc = (src[m], gch[m], win[m], dof[m],
                                     ea_all[m], dstl[m])
        order = np.lexsort((dlc, gc))
        sc, gc, wc, dfc, eac = (sc[order], gc[order], wc[order], dfc[order],
                                eac[order])
        gw = gc * NW + wc
        cnts = np.bincount(gw, minlength=G * NW)
        starts = np.cumsum(cnts) - cnts
        k = np.arange(sc.size) - starts[gw]
        tin = k >> 7
        p = k & 127
        tgi = tile_start[gc, wc] + tin
        blk = block_base[gc] + tgi // BT
        tib = tgi % BT
        slot = blk * EB + tib * TE + p
        tcol = blk * BT + tib

        gidx = np.zeros(NB * EB, np.int16)
        gidx[slot] = (sc - gc * CHUNK).astype(np.int16)
        mtw = np.zeros((NB, 128, EB), BF16NP)
        mtw[blk, dfc, tib * TE + p] = 1.0
        dsto = np.full((128, NB * BT), 255, BF16NP)
        dsto[p, tcol] = dfc.astype(BF16NP)
        eacol = np.zeros((128, NB * BT), BF16NP)
        eacol[p, tcol] = eac.astype(BF16NP)

        im = dict(shared)
        im.update(gidx=_pack_idx16(gidx), mtw=mtw, dsto=dsto, eacol=eacol,
                  xT=np.ascontiguousarray(x[c * NL:(c + 1) * NL].T))
        in_maps.append(im)
    return in_maps, wsched, gob, nbg


# --------------------------------------------------------------- builder ----
def split_sync_waits(nc, max_waits=1):
    for f in nc.m.functions:
        for bb in f.blocks:
            old = bb.instructions
            if not any(i.sync_info and i.sync_info.on_wait
                       and len(i.sync_info.on_wait) > max_waits for i in old):
                continue
            new = []
            for ins in old:
                si = ins.sync_info
                if si is not None and si.on_wait and len(si.on_wait) > max_waits:
                    waits = list(si.on_wait)
                    extra, keep = waits[:-max_waits], waits[-max_waits:]
                    for j, w in enumerate(extra):
                        nop = mybir.InstNoOp(name=f"{ins.name}-wc{j}", ins=[], outs=[])
                        nop.engine = ins.engine
                        nop.sync_info = mybir.SyncInfo(on_wait=[w], on_update=[])
                        new.append(nop)
                    si.on_wait = keep
                new.append(ins)
            bb.instructions = new
    return nc


def build_nc(cfg, wsched, gob, nbg):
    NL, G, CHUNK, TE, BT, EB = cfg.NL, cfg.G, cfg.CHUNK, cfg.TE, cfg.BT, cfg.EB
    HN, L, C, N, NW, NLP = cfg.HN, cfg.L, cfg.C, cfg.N, cfg.NW, cfg.NLP
    NB = int(sum(nbg))
    RG = [list(range(C))]
    # live tile count per block (live tiles form a prefix)
    nlive = [(wsched[b] >= 0).sum() for b in range(NB)]

    nc = bacc.Bacc(None, target_bir_lowering=False, num_swdge_queues=4)
    din = {}

    def ext(name, shape, dt=F32):
        din[name] = nc.dram_tensor(name, shape, dt, kind="ExternalInput")

    ext("xT", [cfg.IN_N, NL])
    ext("gidx", [128, NB * EB // 16], I16)
    ext("mtw", [NB, 128, EB], BF16)
    ext("dsto", [128, NB * BT], BF16)
    ext("eacol", [128, NB * BT], BF16)
    ext("fnw", [cfg.IN_N, HN]); ext("fnb", [HN, 1])
    ext("gam", [HN, 1]); ext("bet", [HN, 1])
    ext("fcw", [HN, L, HN], BF16); ext("fcb", [HN, L])
    ext("wsrc", [HN, L, HN], BF16); ext("wdst", [HN, L, HN], BF16)
    ext("adb", [HN, L])
    ext("u_tiled", [128, L * BT * HN], BF16)
    ext("iota_mod", [128, BT * 128], BF16)
    ext("identb", [128, 128], BF16)
    ext("identf", [128, 128])

    out = nc.dram_tensor("out", [NL, HN], F32, kind="ExternalOutput")
    local_table = nc.dram_tensor("local_table", [NL, 2 * HN], BF16)
    full_table = nc.dram_tensor("full_table", [N, 2 * HN], BF16,
                                addr_space="Shared")
    bn_loc = nc.dram_tensor("bn_loc", [HN, 2], F32)
    bn_sh = nc.dram_tensor("bn_sh", [HN, 2], F32, addr_space="Shared")

    with tile.TileContext(nc) as tc:
        with tc.tile_pool(name="persist", bufs=1) as pp:
            nc.gpsimd.load_library(libcfg.mlp)
            hT = pp.tile([HN, NLP], BF16)
            acc = pp.tile([128, NW, 2 * HN], F32)
            ad_sb = pp.tile([128, NW, HN], BF16)
            w = {}
            for nm, shp, dt in (
                ("fnw", [cfg.IN_N, HN], F32), ("fnb", [HN, 1], F32),
                ("gam", [HN, 1], F32), ("bet", [HN, 1], F32),
                ("fcw", [HN, L, HN], BF16), ("fcb", [HN, L], F32),
                ("wsrc", [HN, L, HN], BF16), ("wdst", [HN, L, HN], BF16),
                ("adb", [HN, L], F32),
                ("iota_mod", [128, BT * 128], BF16),
                ("identb", [128, 128], BF16), ("identf", [128, 128], F32),
                ("dsto", [128, NB * BT], BF16),
                ("eacol", [128, NB * BT], BF16),
            ):
                w[nm] = pp.tile(shp, dt, tag=nm, name=nm)
                nc.sync.dma_start(out=w[nm][:], in_=din[nm][:])
            nc.vector.memset(ad_sb[:], 0.0)

            # ---------------- input + BN ----------------
            with tc.tile_pool(name="bn", bufs=2) as bp, \
                 tc.tile_pool(name="bnc", bufs=3) as bq, \
                 tc.tile_pool(name="bnp", bufs=2, space="PSUM") as bpp:
                st = bp.tile([HN, 2], F32, tag="st")
                nc.vector.memset(st[:], 0.0)
                for ci in range(-(-NL // 512)):
                    c0 = ci * 512
                    n = min(512, NL - c0)
                    xt = bq.tile([cfg.IN_N, 512], F32, tag="xt")
                    nc.sync.dma_start(out=xt[:, :n], in_=din["xT"][:, c0:c0 + n])
                    ps = bpp.tile([HN, 512], F32, tag="ps")
                    nc.tensor.matmul(out=ps[:, :n], lhsT=w["fnw"][:],
                                     rhs=xt[:, :n], start=True, stop=True)
                    nc.vector.tensor_scalar_add(hT[:, c0:c0 + n], ps[:, :n],
                                                w["fnb"][:])
                    stc = bq.tile([HN, 2], F32, tag="stc")
                    nc.vector.reduce_sum(stc[:, 0:1], hT[:, c0:c0 + n],
                                         axis=mybir.AxisListType.X)
                    sq = bq.tile([HN, 512], F32, tag="sq")
                    nc.vector.scalar_tensor_tensor(
                        out=sq[:, :n], in0=hT[:, c0:c0 + n], scalar=1.0,
                        in1=hT[:, c0:c0 + n], op0=AX.mult, op1=AX.mult,
                        accum_out=stc[:, 1:2])
                    nc.vector.tensor_add(st[:], st[:], stc[:])
                nc.sync.dma_start(out=bn_loc[:], in_=st[:])
                nc.gpsimd.collective_compute("AllReduce", AX.add,
                                             replica_groups=RG,
                                             ins=[bn_loc[:]], outs=[bn_sh[:]])
                sg = bp.tile([HN, 2], F32, tag="sg")
                nc.sync.dma_start(out=sg[:], in_=bn_sh[:])
                mean = bp.tile([HN, 1], F32, tag="mean")
                var = bp.tile([HN, 1], F32, tag="var")
                nc.vector.tensor_scalar_mul(mean[:], sg[:, 0:1], 1.0 / N)
                nc.vector.tensor_scalar_mul(var[:], sg[:, 1:2], 1.0 / N)
                msq = bp.tile([HN, 1], F32, tag="msq")
                nc.vector.tensor_mul(msq[:], mean[:], mean[:])
                nc.vector.tensor_sub(var[:], var[:], msq[:])
                nc.vector.tensor_scalar_add(var[:], var[:], cfg.EPS)
                rs = bp.tile([HN, 1], F32, tag="rs")
                nc.scalar.activation(out=rs[:], in_=var[:], func=AF.Sqrt)
                nc.vector.reciprocal(rs[:], rs[:])
                scale = bp.tile([HN, 1], F32, tag="scale")
                nc.vector.tensor_mul(scale[:], rs[:], w["gam"][:])
                nbias = bp.tile([HN, 1], F32, tag="nbias")
                nc.vector.tensor_mul(nbias[:], mean[:], scale[:])
                nc.vector.scalar_tensor_tensor(out=nbias[:], in0=nbias[:],
                                               scalar=-1.0, in1=w["bet"][:],
                                               op0=AX.mult, op1=AX.add)
                nc.vector.tensor_scalar(out=hT[:, :NL], in0=hT[:, :NL],
                                        scalar1=scale[:], scalar2=nbias[:],
                                        op0=AX.mult, op1=AX.add)

            # ---------------- layers ----------------
            for l in range(L):
                # table build: local_table rows [a_s|hl], ad_sb windows
                with tc.tile_pool(name=f"tb{l}", bufs=2) as tp, \
                     tc.tile_pool(name=f"tp{l}", bufs=2, space="PSUM") as tpp:
                    for ci in range(-(-NL // 512)):
                        c0 = ci * 512
                        n = min(512, NL - c0)
                        psh = tpp.tile([HN, 512], F32, tag="psh")
                        nc.tensor.matmul(out=psh[:, :n], lhsT=w["fcw"][:, l, :],
                                         rhs=hT[:, c0:c0 + n], start=True, stop=True)
                        hl = tp.tile([HN, 512], BF16, tag="hl")
                        nc.vector.tensor_scalar_add(hl[:, :n], psh[:, :n],
                                                    w["fcb"][:, l:l + 1])
                        pss = tpp.tile([HN, 512], F32, tag="psh")
                        nc.tensor.matmul(out=pss[:, :n], lhsT=w["wsrc"][:, l, :],
                                         rhs=hl[:, :n], start=True, stop=True)
                        asb = tp.tile([HN, 512], BF16, tag="asb")
                        nc.scalar.copy(out=asb[:, :n], in_=pss[:, :n])
                        psd = tpp.tile([HN, 512], F32, tag="psh")
                        nc.tensor.matmul(out=psd[:, :n], lhsT=w["wdst"][:, l, :],
                                         rhs=hl[:, :n], start=True, stop=True)
                        adt = tp.tile([HN, 512], BF16, tag="adt")
                        nc.vector.tensor_scalar_add(adt[:, :n], psd[:, :n],
                                                    w["adb"][:, l:l + 1])
                        for si in range(-(-n // 128)):
                            s0, sn = si * 128, min(128, n - si * 128)
                            widx = (c0 + si * 128) // 128
                            rows = tp.tile([128, 2 * HN], BF16, tag="rows")
                            pt = tpp.tile([128, HN], BF16, tag="pt")
                            nc.tensor.transpose(out=pt[:sn, :],
                                                in_=asb[:, s0:s0 + sn],
                                                identity=w["identb"][:HN, :HN])
                            nc.scalar.copy(out=rows[:sn, 0:HN], in_=pt[:sn, :])
                            pt2 = tpp.tile([128, HN], BF16, tag="pt")
                            nc.tensor.transpose(out=pt2[:sn, :],
                                                in_=hl[:, s0:s0 + sn],
                                                identity=w["identb"][:HN, :HN])
                            nc.scalar.copy(out=rows[:sn, HN:2 * HN], in_=pt2[:sn, :])
                            nc.sync.dma_start(
                                out=local_table[c0 + s0:c0 + s0 + sn, :],
                                in_=rows[:sn, :])
                            pt3 = tpp.tile([128, HN], BF16, tag="pt")
                            nc.tensor.transpose(out=pt3[:sn, :],
                                                in_=adt[:, s0:s0 + sn],
                                                identity=w["identb"][:HN, :HN])
                            nc.vector.tensor_copy(ad_sb[:sn, widx, :], pt3[:sn, :])

                nc.gpsimd.collective_compute("AllGather", AX.bypass,
                                             replica_groups=RG,
                                             ins=[local_table[:]],
                                             outs=[full_table[:]])
                nc.vector.memset(acc[:], 0.0)

                # ---------------- edge phase ----------------
                with tc.tile_pool(name=f"ep{l}", bufs=2) as ep, \
                     tc.tile_pool(name=f"ut{l}", bufs=1) as up, \
                     tc.tile_pool(name=f"pp{l}", bufs=2, space="PSUM") as epp, \
                     tc.tile_pool(name=f"wp{l}", bufs=4, space="PSUM") as wpp:
                    u_l = up.tile([128, BT * HN], BF16, tag="u_l")
                    nc.sync.dma_start(
                        out=u_l[:],
                        in_=din["u_tiled"][:, l * BT * HN:(l + 1) * BT * HN])
                    for b in range(NB):
                        g = int(gob[b])
                        nl_b = int(nlive[b])
                        gix = ep.tile([128, EB // 16], I16, tag="gix")
                        nc.sync.dma_start(
                            out=gix[:],
                            in_=din["gidx"][:, b * (EB // 16):(b + 1) * (EB // 16)])
                        mtw_t = ep.tile([128, EB], BF16, tag="mtw_t")
                        nc.sync.dma_start(out=mtw_t[:], in_=din["mtw"][b, :, :])
                        srcr = ep.tile([128, BT, 2 * HN], BF16, tag="srcr")
                        for hh in range(2):
                            nc.gpsimd.dma_gather(
                                out_ap=srcr[:, hh * (BT // 2):(hh + 1) * (BT // 2), :],
                                in_ap=full_table[g * CHUNK:(g + 1) * CHUNK, :],
                                idxs_ap=gix[:, hh * (EB // 32):(hh + 1) * (EB // 32)],
                                num_idxs=EB // 2, num_idxs_reg=EB // 2,
                                elem_size=2 * HN,
                                single_packet=False, queue_num=(b + 2 * hh) % 4)
                        mt_ew = ep.tile([128, BT, 128], BF16, tag="mt_ew")
                        nc.vector.tensor_tensor(
                            out=mt_ew[:],
                            in0=w["dsto"][:, b * BT:(b + 1) * BT].rearrange(
                                "p (t o) -> p t o", o=1).to_broadcast([128, BT, 128]),
                            in1=w["iota_mod"][:].rearrange(
                                "p (t s) -> p t s", s=128),
                            op=AX.is_equal)
                        eau = ep.tile([128, BT, HN], BF16, tag="eau")
                        nc.vector.tensor_tensor(
                            out=eau[:],
                            in0=u_l[:].rearrange("p (t h) -> p t h", h=HN),
                            in1=w["eacol"][:, b * BT:(b + 1) * BT].rearrange(
                                "p (t o) -> p t o", o=1).to_broadcast([128, BT, HN]),
                            op=AX.mult)
                        alpha = ep.tile([128, BT, HN], F32, tag="alpha")
                        contrib = ep.tile([128, BT, 2 * HN], BF16, tag="contrib")
                        for grp in range(-(-nl_b // 8)):
                            t0 = grp * 8
                            gn = min(8, nl_b - t0)
                            psx = epp.tile([128, 8 * HN], F32, tag="psx")
                            for t2 in range(gn):
                                t = t0 + t2
                                nc.tensor.matmul(
                                    out=psx[:, t2 * HN:(t2 + 1) * HN],
                                    lhsT=mtw_t[:, t * TE:(t + 1) * TE],
                                    rhs=ad_sb[:, int(wsched[b, t]), :],
                                    start=True, stop=True)
                            nc.vector.tensor_tensor(
                                out=alpha[:, t0:t0 + gn, :],
                                in0=psx[:, :gn * HN].rearrange(
                                    "p (t h) -> p t h", h=HN),
                                in1=srcr[:, t0:t0 + gn, 0:HN],
                                op=AX.add)
                            nc.vector.tensor_tensor(
                                out=alpha[:, t0:t0 + gn, :],
                                in0=alpha[:, t0:t0 + gn, :],
                                in1=eau[:, t0:t0 + gn, :],
                                op=AX.add)
                        nc.vector.scalar_tensor_tensor(
                            out=alpha[:, :nl_b, :], in0=alpha[:, :nl_b, :],
                            scalar=cfg.NEG, in1=alpha[:, :nl_b, :],
                            op0=AX.mult, op1=AX.max)
                        nc.scalar.activation(out=contrib[:, :nl_b, HN:2 * HN],
                                             in_=alpha[:, :nl_b, :], func=AF.Exp)
                        nc.vector.tensor_mul(contrib[:, :nl_b, 0:HN],
                                             contrib[:, :nl_b, HN:2 * HN],
                                             srcr[:, :nl_b, HN:2 * HN])
                        # aggregation: window runs -> PSUM accumulate -> acc
                        t = 0
                        while t < nl_b:
                            wv = int(wsched[b, t])
                            t1 = t
                            while t1 + 1 < nl_b and wsched[b, t1 + 1] == wv:
                                t1 += 1
                            wps = wpp.tile([128, 2 * HN], F32, tag="wps")
                            for tt in range(t, t1 + 1):
                                nc.tensor.matmul(
                                    out=wps[:], lhsT=mt_ew[:, tt, :],
                                    rhs=contrib[:, tt, :],
                                    start=(tt == t), stop=(tt == t1))
                            nc.vector.tensor_add(acc[:, wv, :], acc[:, wv, :],
                                                 wps[:])
                            t = t1 + 1

                # ---------------- finalize ----------------
                with tc.tile_pool(name=f"fi{l}", bufs=3) as fp, \
                     tc.tile_pool(name=f"fp{l}", bufs=3, space="PSUM") as fpp:
                    if l < L - 1:
                        for wv in range(NW):
                            ptr = fpp.tile([128, 128], F32, tag="ptr")
                            nc.tensor.transpose(out=ptr[:], in_=acc[:, wv, :],
                                                identity=w["identf"][:])
                            den = fp.tile([HN, 128], F32, tag="den")
                            nc.vector.tensor_scalar_add(den[:], ptr[HN:2 * HN, :],
                                                        1e-16)
                            nc.vector.reciprocal(den[:], den[:])
                            nc.vector.tensor_mul(hT[:, wv * 128:(wv + 1) * 128],
                                                 ptr[0:HN, :], den[:])
                    else:
                        for wv in range(NW):
                            n = min(128, NL - wv * 128)
                            den = fp.tile([128, HN], F32, tag="den2")
                            nc.vector.tensor_scalar_add(
                                den[:n, :], acc[:n, wv, HN:2 * HN], 1e-16)
                            nc.vector.reciprocal(den[:n, :], den[:n, :])
                            orow = fp.tile([128, HN], F32, tag="orow")
                            nc.vector.tensor_mul(orow[:n, :], acc[:n, wv, 0:HN],
                                                 den[:n, :])
                            nc.sync.dma_start(out=out[wv * 128:wv * 128 + n, :],
                                              in_=orow[:n, :])
    return nc


# --------------------------------------------------------------- entry ------
def kernel(**inputs):
    cfg = Cfg()
    in_maps, wsched, gob, nbg = host_prepare(cfg, inputs)
    nc = build_nc(cfg, wsched, gob, nbg)
    nc.compile()
    split_sync_waits(nc)
    res = run_bass_kernel_spmd(nc, in_maps, core_ids=list(range(cfg.C)),
                               trace=bool(int(os.environ.get("GNN_TRACE", "0"))))
    if res.exec_time_ns is not None:
        print(f"HW exec time: {res.exec_time_ns} ns")
    if res.instructions_and_trace is not None:
        print(f"trace: {res.instructions_and_trace[1]}")
    h = np.concatenate([np.asarray(res.results[c]["out"])
                        for c in range(cfg.C)], axis=0)
    return h.reshape(cfg.B, -1, cfg.HN)


# revision 11
# speedup vs baseline: 1.6016x; 1.0752x over previous
"""GNN message-passing encoder (GAT-style) on 8 Trainium2 NeuronCores.

Self-contained: hardcodes the problem shapes (N=100000, E=1600000, HN=64, L=3).

Sharding: core c owns dst nodes [c*NL, (c+1)*NL). Edges sorted by
(src_chunk g, dst); dst space split into 128-aligned windows; per (g, window)
the tile count T_gw is the max over cores (shared compile-time schedule).

Per layer, on device (all bf16 tables):
- table build: hlT = fc_w[l]^T h; a_s = W_src hl; a_d = W_dst hl + bias;
  node rows [a_s|hl] -> local_table -> AllGather full_table (256B rows);
  a_d rows transposed into SBUF-resident ad_sb[128, NW, 64].
- per block (4096 edges): dma_gather 4096 src rows (bf16, 1 desc each);
  host-built one-hot mtw [128 dstoff x 4096 e] from HBM; expand matmul
  psx = mtw^T @ ad_sb[:, w, :] per tile -> per-edge a_d; alpha = psx + a_s
  + ea*u (broadcast DVE); LRelu; Exp -> contrib [ex*hl | ex] bf16;
  aggregation matmul per tile accumulates into per-window PSUM
  (mt_ew one-hot built on-device); window flush adds into SBUF acc (f32).
- finalize from SBUF acc: h = num/(den+1e-16) -> hT bf16 (or out rows f32).
No HBM scatter/accumulators; only per-edge gather remains indexed.
"""
import os
import sys

for _p in ("/opt/trn_rl_repo",):
    if _p not in sys.path:
        sys.path.insert(0, _p)

import numpy as np
import ml_dtypes

import concourse.bass as bass
import concourse.mybir as mybir
import concourse.tile as tile
from concourse import bacc
from concourse import library_config as libcfg
from concourse.bass_utils import run_bass_kernel_spmd

F32 = mybir.dt.float32
BF16 = mybir.dt.bfloat16
I16 = mybir.dt.int16
AX = mybir.AluOpType
AF = mybir.ActivationFunctionType
BF16NP = ml_dtypes.bfloat16


class Cfg:
    def __init__(self):
        self.N, self.E, self.C = 100000, 1600000, 8
        self.HN, self.IN_N, self.L, self.B = 64, 3, 3, 10
        self.NEG, self.EPS = 0.2, 1e-5
        self.NL = self.N // self.C          # 12500
        self.CHUNK = 25000
        self.G = self.N // self.CHUNK       # 4
        self.TE = 128
        self.BT = 32
        self.EB = self.TE * self.BT         # 4096
        self.NW = -(-self.NL // 128)        # 98
        self.NLP = self.NW * 128            # 12544


# ------------------------------------------------------------- host prep ----
def _pack_idx16(vals):
    v = np.asarray(vals, dtype=np.int16)
    assert v.size % 16 == 0
    return np.tile(v.reshape(-1, 16).T, (8, 1)).copy()


def host_prepare(cfg, inputs):
    C, G, NL, CHUNK, NW = cfg.C, cfg.G, cfg.NL, cfg.CHUNK, cfg.NW
    TE, BT, EB, HN, L = cfg.TE, cfg.BT, cfg.EB, cfg.HN, cfg.L

    ei = np.asarray(inputs["edge_index"]).astype(np.int64)
    src, dst = ei[0], ei[1]
    ea_all = np.asarray(inputs["edge_attr"], np.float32)[:, 0]
    core = dst // NL
    dstl = dst - core * NL
    gch = src // CHUNK
    win = dstl >> 7
    dof = dstl & 127

    # shared tile schedule: T_gw = max over cores of ceil(cnt/128)
    key = (core * G + gch) * NW + win
    cnt = np.bincount(key, minlength=C * G * NW).reshape(C, G, NW)
    Tgw = -(-cnt.max(axis=0) // TE)                    # [G, NW]
    tiles_g = Tgw.sum(axis=1)                          # [G]
    nbg = np.maximum(1, -(-tiles_g // BT))
    NB = int(nbg.sum())
    tile_start = np.cumsum(Tgw, axis=1) - Tgw          # [G, NW]
    block_base = np.concatenate([[0], np.cumsum(nbg)])[:G]

    # per-block window schedule (window id per tile slot, -1 = padding)
    wsched = -np.ones((NB, BT), np.int64)
    for g in range(G):
        flat = np.repeat(np.arange(NW), Tgw[g])
        b = block_base[g] + np.arange(len(flat)) // BT
        wsched[b, np.arange(len(flat)) % BT] = flat
    gob = np.repeat(np.arange(G), nbg)

    # weights
    fnw = np.asarray(inputs["fc_node_w"], np.float32)
    fnb = np.asarray(inputs["fc_node_b"], np.float32)
    few = np.asarray(inputs["fc_edge_w"], np.float32)
    feb = np.asarray(inputs["fc_edge_b"], np.float32)
    gam = np.asarray(inputs["bn_gamma"], np.float32)
    bet = np.asarray(inputs["bn_beta"], np.float32)
    fcw = np.asarray(inputs["fc_w"], np.float32)
    fcb = np.asarray(inputs["fc_b"], np.float32)
    aw = np.asarray(inputs["attn_w"], np.float32)
    ab = np.asarray(inputs["attn_b"], np.float32)
    wdst, wsrc, we = aw[:, :HN, :], aw[:, HN:2 * HN, :], aw[:, 2 * HN:, :]
    u = np.stack([few[0] @ we[l] for l in range(L)])          # [L, 64]
    adb = np.stack([ab[l] + feb @ we[l] for l in range(L)])   # [L, 64]
    x = np.asarray(inputs["x"], np.float32)

    shared = dict(
        fnw=fnw, fnb=fnb.reshape(-1, 1), gam=gam.reshape(-1, 1),
        bet=bet.reshape(-1, 1),
        fcw=np.ascontiguousarray(fcw.transpose(1, 0, 2)).astype(BF16NP),
        wsrc=np.ascontiguousarray(wsrc.transpose(1, 0, 2)).astype(BF16NP),
        wdst=np.ascontiguousarray(wdst.transpose(1, 0, 2)).astype(BF16NP),
        fcb=np.ascontiguousarray(fcb.T), adb=np.ascontiguousarray(adb.T),
        u_row=u.reshape(1, -1).astype(BF16NP),
        identb=np.eye(128, dtype=BF16NP),
        identf=np.eye(128, dtype=np.float32),
    )

    in_maps = []
    for c in range(C):
        m = core == c
        sc, gc, wc, dfc, eac, dlc = (src[m], gch[m], win[m], dof[m],
                                     ea_all[m], dstl[m])
        order = np.lexsort((dlc, gc))
        sc, gc, wc, dfc, eac = (sc[order], gc[order], wc[order], dfc[order],
                                eac[order])
        gw = gc * NW + wc
        cnts = np.bincount(gw, minlength=G * NW)
        starts = np.cumsum(cnts) - cnts
        k = np.arange(sc.size) - starts[gw]
        tin = k >> 7
        p = k & 127
        tgi = tile_start[gc, wc] + tin
        blk = block_base[gc] + tgi // BT
        tib = tgi % BT
        slot = blk * EB + tib * TE + p

        gidx = np.zeros(NB * EB, np.int16)
        gidx[slot] = (sc - gc * CHUNK).astype(np.int16)
        mtw = np.zeros((NB, 128, EB), BF16NP)
        mtw[blk, dfc, tib * TE + p] = 1.0
        mte = np.zeros((NB, 128, EB), BF16NP)
        mte[blk, p, tib * TE + dfc] = 1.0
        eab = np.zeros((NB, EB), BF16NP)
        eab[blk, tib * TE + p] = eac.astype(BF16NP)

        im = dict(shared)
        im.update(gidx=_pack_idx16(gidx), mtw=mtw, mte=mte, eab=eab,
                  xT=np.ascontiguousarray(x[c * NL:(c + 1) * NL].T))
        in_maps.append(im)
    return in_maps, wsched, gob, nbg


# --------------------------------------------------------------- builder ----
def split_sync_waits(nc, max_waits=1):
    for f in nc.m.functions:
        for bb in f.blocks:
            old = bb.instructions
            if not any(i.sync_info and i.sync_info.on_wait
                       and len(i.sync_info.on_wait) > max_waits for i in old):
                continue
            new = []
            for ins in old:
                si = ins.sync_info
                if si is not None and si.on_wait and len(si.on_wait) > max_waits:
                    waits = list(si.on_wait)
                    extra, keep = waits[:-max_waits], waits[-max_waits:]
                    for j, w in enumerate(extra):
                        nop = mybir.InstNoOp(name=f"{ins.name}-wc{j}", ins=[], outs=[])
                        nop.engine = ins.engine
                        nop.sync_info = mybir.SyncInfo(on_wait=[w], on_update=[])
                        new.append(nop)
                    si.on_wait = keep
                new.append(ins)
            bb.instructions = new
    return nc


def build_nc(cfg, wsched, gob, nbg):
    NL, G, CHUNK, TE, BT, EB = cfg.NL, cfg.G, cfg.CHUNK, cfg.TE, cfg.BT, cfg.EB
    HN, L, C, N, NW, NLP = cfg.HN, cfg.L, cfg.C, cfg.N, cfg.NW, cfg.NLP
    NB = int(sum(nbg))
    RG = [list(range(C))]
    # live tile count per block (live tiles form a prefix)
    nlive = [(wsched[b] >= 0).sum() for b in range(NB)]

    nc = bacc.Bacc(None, target_bir_lowering=False, num_swdge_queues=4)
    din = {}

    def ext(name, shape, dt=F32):
        din[name] = nc.dram_tensor(name, shape, dt, kind="ExternalInput")

    ext("xT", [cfg.IN_N, NL])
    ext("gidx", [128, NB * EB // 16], I16)
    ext("mtw", [NB, 128, EB], BF16)
    ext("mte", [NB, 128, EB], BF16)
    ext("eab", [NB, EB], BF16)
    ext("fnw", [cfg.IN_N, HN]); ext("fnb", [HN, 1])
    ext("gam", [HN, 1]); ext("bet", [HN, 1])
    ext("fcw", [HN, L, HN], BF16); ext("fcb", [HN, L])
    ext("wsrc", [HN, L, HN], BF16); ext("wdst", [HN, L, HN], BF16)
    ext("adb", [HN, L])
    ext("u_row", [1, L * HN], BF16)
    ext("identb", [128, 128], BF16)
    ext("identf", [128, 128])

    out = nc.dram_tensor("out", [NL, HN], F32, kind="ExternalOutput")
    local_table = nc.dram_tensor("local_table", [NL, 2 * HN], BF16)
    full_table = nc.dram_tensor("full_table", [N, 2 * HN], BF16,
                                addr_space="Shared")
    bn_loc = nc.dram_tensor("bn_loc", [HN, 2], F32)
    bn_sh = nc.dram_tensor("bn_sh", [HN, 2], F32, addr_space="Shared")

    with tile.TileContext(nc) as tc:
        with tc.tile_pool(name="persist", bufs=1) as pp:
            nc.gpsimd.load_library(libcfg.mlp)
            hT = pp.tile([HN, NLP], BF16)
            acc = pp.tile([128, NW, 2 * HN], F32)
            ad_sb = pp.tile([128, NW, HN], BF16)
            w = {}
            for nm, shp, dt in (
                ("fnw", [cfg.IN_N, HN], F32), ("fnb", [HN, 1], F32),
                ("gam", [HN, 1], F32), ("bet", [HN, 1], F32),
                ("fcw", [HN, L, HN], BF16), ("fcb", [HN, L], F32),
                ("wsrc", [HN, L, HN], BF16), ("wdst", [HN, L, HN], BF16),
                ("adb", [HN, L], F32),
                ("u_row", [1, L * HN], BF16),
                ("identb", [128, 128], BF16), ("identf", [128, 128], F32),
            ):
                w[nm] = pp.tile(shp, dt, tag=nm, name=nm)
                nc.sync.dma_start(out=w[nm][:], in_=din[nm][:])
            nc.vector.memset(ad_sb[:], 0.0)

            # ---------------- input + BN ----------------
            with tc.tile_pool(name="bn", bufs=2) as bp, \
                 tc.tile_pool(name="bnc", bufs=3) as bq, \
                 tc.tile_pool(name="bnp", bufs=2, space="PSUM") as bpp:
                st = bp.tile([HN, 2], F32, tag="st")
                nc.vector.memset(st[:], 0.0)
                for ci in range(-(-NL // 512)):
                    c0 = ci * 512
                    n = min(512, NL - c0)
                    xt = bq.tile([cfg.IN_N, 512], F32, tag="xt")
                    nc.sync.dma_start(out=xt[:, :n], in_=din["xT"][:, c0:c0 + n])
                    ps = bpp.tile([HN, 512], F32, tag="ps")
                    nc.tensor.matmul(out=ps[:, :n], lhsT=w["fnw"][:],
                                     rhs=xt[:, :n], start=True, stop=True)
                    nc.vector.tensor_scalar_add(hT[:, c0:c0 + n], ps[:, :n],
                                                w["fnb"][:])
                    stc = bq.tile([HN, 2], F32, tag="stc")
                    nc.vector.reduce_sum(stc[:, 0:1], hT[:, c0:c0 + n],
                                         axis=mybir.AxisListType.X)
                    sq = bq.tile([HN, 512], F32, tag="sq")
                    nc.vector.scalar_tensor_tensor(
                        out=sq[:, :n], in0=hT[:, c0:c0 + n], scalar=1.0,
                        in1=hT[:, c0:c0 + n], op0=AX.mult, op1=AX.mult,
                        accum_out=stc[:, 1:2])
                    nc.vector.tensor_add(st[:], st[:], stc[:])
                nc.sync.dma_start(out=bn_loc[:], in_=st[:])
                nc.gpsimd.collective_compute("AllReduce", AX.add,
                                             replica_groups=RG,
                                             ins=[bn_loc[:]], outs=[bn_sh[:]])
                sg = bp.tile([HN, 2], F32, tag="sg")
                nc.sync.dma_start(out=sg[:], in_=bn_sh[:])
                mean = bp.tile([HN, 1], F32, tag="mean")
                var = bp.tile([HN, 1], F32, tag="var")
                nc.vector.tensor_scalar_mul(mean[:], sg[:, 0:1], 1.0 / N)
                nc.vector.tensor_scalar_mul(var[:], sg[:, 1:2], 1.0 / N)
                msq = bp.tile([HN, 1], F32, tag="msq")
                nc.vector.tensor_mul(msq[:], mean[:], mean[:])
                nc.vector.tensor_sub(var[:], var[:], msq[:])
                nc.vector.tensor_scalar_add(var[:], var[:], cfg.EPS)
                rs = bp.tile([HN, 1], F32, tag="rs")
                nc.scalar.activation(out=rs[:], in_=var[:], func=AF.Sqrt)
                nc.vector.reciprocal(rs[:], rs[:])
                scale = bp.tile([HN, 1], F32, tag="scale")
                nc.vector.tensor_mul(scale[:], rs[:], w["gam"][:])
                nbias = bp.tile([HN, 1], F32, tag="nbias")
                nc.vector.tensor_mul(nbias[:], mean[:], scale[:])
                nc.vector.scalar_tensor_tensor(out=nbias[:], in0=nbias[:],
                                               scalar=-1.0, in1=w["bet"][:],
                                               op0=AX.mult, op1=AX.add)
                nc.vector.tensor_scalar(out=hT[:, :NL], in0=hT[:, :NL],
                                        scalar1=scale[:], scalar2=nbias[:],
                                        op0=AX.mult, op1=AX.add)

            # ---------------- layers ----------------
            for l in range(L):
                # per-window: finalize layer l-1 (l>0) + table build
                with tc.tile_pool(name=f"tb{l}", bufs=3) as tp, \
                     tc.tile_pool(name=f"tp{l}", bufs=2, space="PSUM") as tpp:
                    for wv in range(NW):
                        c0 = wv * 128
                        n = min(128, NL - c0)
                        if l > 0:
                            ptr = tpp.tile([128, 128], F32, tag="ptr")
                            nc.tensor.transpose(out=ptr[:], in_=acc[:, wv, :],
                                                identity=w["identf"][:])
                            den = tp.tile([HN, 128], F32, tag="den")
                            nc.vector.tensor_scalar_add(den[:], ptr[HN:2 * HN, :],
                                                        1e-16)
                            nc.vector.reciprocal(den[:], den[:])
                            nc.vector.tensor_mul(hT[:, c0:c0 + 128],
                                                 ptr[0:HN, :], den[:])
                        psh = tpp.tile([HN, 128], F32, tag="psh")
                        nc.tensor.matmul(out=psh[:, :n], lhsT=w["fcw"][:, l, :],
                                         rhs=hT[:, c0:c0 + n], start=True, stop=True)
                        hl = tp.tile([HN, 128], BF16, tag="hl")
                        nc.vector.tensor_scalar_add(hl[:, :n], psh[:, :n],
                                                    w["fcb"][:, l:l + 1])
                        pss = tpp.tile([HN, 128], F32, tag="psh")
                        nc.tensor.matmul(out=pss[:, :n], lhsT=w["wsrc"][:, l, :],
                                         rhs=hl[:, :n], start=True, stop=True)
                        asb = tp.tile([HN, 128], BF16, tag="asb")
                        nc.scalar.copy(out=asb[:, :n], in_=pss[:, :n])
                        psd = tpp.tile([HN, 128], F32, tag="psh")
                        nc.tensor.matmul(out=psd[:, :n], lhsT=w["wdst"][:, l, :],
                                         rhs=hl[:, :n], start=True, stop=True)
                        adt = tp.tile([HN, 128], BF16, tag="adt")
                        nc.vector.tensor_scalar_add(adt[:, :n], psd[:, :n],
                                                    w["adb"][:, l:l + 1])
                        rows = tp.tile([128, 2 * HN], BF16, tag="rows")
                        pt = tpp.tile([128, HN], BF16, tag="pt")
                        nc.tensor.transpose(out=pt[:n, :], in_=asb[:, :n],
                                            identity=w["identb"][:HN, :HN])
                        nc.scalar.copy(out=rows[:n, 0:HN], in_=pt[:n, :])
                        pt2 = tpp.tile([128, HN], BF16, tag="pt")
                        nc.tensor.transpose(out=pt2[:n, :], in_=hl[:, :n],
                                            identity=w["identb"][:HN, :HN])
                        nc.scalar.copy(out=rows[:n, HN:2 * HN], in_=pt2[:n, :])
                        nc.sync.dma_start(out=local_table[c0:c0 + n, :],
                                          in_=rows[:n, :])
                        pt3 = tpp.tile([128, HN], BF16, tag="pt")
                        nc.tensor.transpose(out=pt3[:n, :], in_=adt[:, :n],
                                            identity=w["identb"][:HN, :HN])
                        nc.vector.tensor_copy(ad_sb[:n, wv, :], pt3[:n, :])

                nc.gpsimd.collective_compute("AllGather", AX.bypass,
                                             replica_groups=RG,
                                             ins=[local_table[:]],
                                             outs=[full_table[:]])
                nc.vector.memset(acc[:], 0.0)

                # ---------------- edge phase ----------------
                with tc.tile_pool(name=f"ep{l}", bufs=2) as ep, \
                     tc.tile_pool(name=f"pp{l}", bufs=2, space="PSUM") as epp, \
                     tc.tile_pool(name=f"wp{l}", bufs=4, space="PSUM") as wpp:
                    for b in range(NB):
                        g = int(gob[b])
                        nl_b = int(nlive[b])
                        gix = ep.tile([128, EB // 16], I16, tag="gix")
                        nc.sync.dma_start(
                            out=gix[:],
                            in_=din["gidx"][:, b * (EB // 16):(b + 1) * (EB // 16)])
                        mtw_t = ep.tile([128, EB], BF16, tag="mtw_t")
                        nc.sync.dma_start(out=mtw_t[:], in_=din["mtw"][b, :, :])
                        mt_ew = ep.tile([128, BT, 128], BF16, tag="mt_ew")
                        nc.sync.dma_start(
                            out=mt_ew[:],
                            in_=din["mte"][b, :, :].rearrange(
                                "p (t s) -> p t s", s=128))
                        eat = ep.tile([1, EB], BF16, tag="eat")
                        nc.sync.dma_start(out=eat[:], in_=din["eab"][b:b + 1, :])
                        srcr = ep.tile([128, BT, 2 * HN], BF16, tag="srcr")
                        for hh in range(4):
                            nc.gpsimd.dma_gather(
                                out_ap=srcr[:, hh * (BT // 4):(hh + 1) * (BT // 4), :],
                                in_ap=full_table[g * CHUNK:(g + 1) * CHUNK, :],
                                idxs_ap=gix[:, hh * (EB // 64):(hh + 1) * (EB // 64)],
                                num_idxs=EB // 4, num_idxs_reg=EB // 4,
                                elem_size=2 * HN,
                                single_packet=True, queue_num=(b + hh) % 4)
                        alpha = ep.tile([128, BT, HN], F32, tag="alpha")
                        contrib = ep.tile([128, BT, 2 * HN], BF16, tag="contrib")
                        for grp in range(-(-nl_b // 8)):
                            t0 = grp * 8
                            gn = min(8, nl_b - t0)
                            psx = epp.tile([128, 8 * HN], F32, tag="psx")
                            for t2 in range(gn):
                                t = t0 + t2
                                nc.tensor.matmul(
                                    out=psx[:, t2 * HN:(t2 + 1) * HN],
                                    lhsT=mtw_t[:, t * TE:(t + 1) * TE],
                                    rhs=ad_sb[:, int(wsched[b, t]), :],
                                    start=True, stop=False)
                                nc.tensor.matmul(
                                    out=psx[:, t2 * HN:(t2 + 1) * HN],
                                    lhsT=eat[0:1, t * TE:(t + 1) * TE],
                                    rhs=w["u_row"][:, l * HN:(l + 1) * HN],
                                    start=False, stop=True)
                            nc.vector.tensor_tensor(
                                out=alpha[:, t0:t0 + gn, :],
                                in0=psx[:, :gn * HN].rearrange(
                                    "p (t h) -> p t h", h=HN),
                                in1=srcr[:, t0:t0 + gn, 0:HN],
                                op=AX.add)
                        extmp = ep.tile([128, BT, HN], BF16, tag="extmp")
                        nc.scalar.activation(out=contrib[:, :nl_b, HN:2 * HN],
                                             in_=alpha[:, :nl_b, :], func=AF.Exp)
                        nc.scalar.activation(out=extmp[:, :nl_b, :],
                                             in_=alpha[:, :nl_b, :], func=AF.Exp,
                                             scale=cfg.NEG)
                        nc.vector.tensor_tensor(
                            out=contrib[:, :nl_b, HN:2 * HN],
                            in0=contrib[:, :nl_b, HN:2 * HN],
                            in1=extmp[:, :nl_b, :], op=AX.max)
                        nc.vector.tensor_mul(contrib[:, :nl_b, 0:HN],
                                             contrib[:, :nl_b, HN:2 * HN],
                                             srcr[:, :nl_b, HN:2 * HN])
                        # aggregation: window runs -> PSUM accumulate -> acc
                        t = 0
                        while t < nl_b:
                            wv = int(wsched[b, t])
                            t1 = t
                            while t1 + 1 < nl_b and wsched[b, t1 + 1] == wv:
                                t1 += 1
                            wps = wpp.tile([128, 2 * HN], F32, tag="wps")
                            for tt in range(t, t1 + 1):
                                nc.tensor.matmul(
                                    out=wps[:], lhsT=mt_ew[:, tt, :],
                                    rhs=contrib[:, tt, :],
                                    start=(tt == t), stop=(tt == t1))
                            nc.vector.tensor_add(acc[:, wv, :], acc[:, wv, :],
                                                 wps[:])
                            t = t1 + 1

            # ---------------- output ----------------
            with tc.tile_pool(name="fo", bufs=3) as fp:
                for wv in range(NW):
                    n = min(128, NL - wv * 128)
                    den = fp.tile([128, HN], F32, tag="den2")
                    nc.vector.tensor_scalar_add(
                        den[:n, :], acc[:n, wv, HN:2 * HN], 1e-16)
                    nc.vector.reciprocal(den[:n, :], den[:n, :])
                    orow = fp.tile([128, HN], F32, tag="orow")
                    nc.vector.tensor_mul(orow[:n, :], acc[:n, wv, 0:HN],
                                         den[:n, :])
                    nc.sync.dma_start(out=out[wv * 128:wv * 128 + n, :],
                                      in_=orow[:n, :])
    return nc


# --------------------------------------------------------------- entry ------
def kernel(**inputs):
    cfg = Cfg()
    in_maps, wsched, gob, nbg = host_prepare(cfg, inputs)
    nc = build_nc(cfg, wsched, gob, nbg)
    nc.compile()
    split_sync_waits(nc)
    res = run_bass_kernel_spmd(nc, in_maps, core_ids=list(range(cfg.C)),
                               trace=bool(int(os.environ.get("GNN_TRACE", "0"))))
    if res.exec_time_ns is not None:
        print(f"HW exec time: {res.exec_time_ns} ns")
    if res.instructions_and_trace is not None:
        print(f"trace: {res.instructions_and_trace[1]}")
    h = np.concatenate([np.asarray(res.results[c]["out"])
                        for c in range(cfg.C)], axis=0)
    return h.reshape(cfg.B, -1, cfg.HN)


# revision 14
# speedup vs baseline: 1.7430x; 1.0883x over previous
"""GNN message-passing encoder (GAT-style) on 8 Trainium2 NeuronCores.

Self-contained: hardcodes the problem shapes (N=100000, E=1600000, HN=64, L=3).

Sharding: core c owns dst nodes [c*NL, (c+1)*NL). Edges sorted by
(src_chunk g, dst); dst space split into 128-aligned windows; per (g, window)
the tile count T_gw is the max over cores (shared compile-time schedule).

Per layer, on device (all bf16 tables):
- table build: hlT = fc_w[l]^T h; a_s = W_src hl; a_d = W_dst hl + bias;
  node rows [a_s|hl] -> local_table -> AllGather full_table (256B rows);
  a_d rows transposed into SBUF-resident ad_sb[128, NW, 64].
- per block (4096 edges): dma_gather 4096 src rows (bf16, 1 desc each);
  host-built one-hot mtw [128 dstoff x 4096 e] from HBM; expand matmul
  psx = mtw^T @ ad_sb[:, w, :] per tile -> per-edge a_d; alpha = psx + a_s
  + ea*u (broadcast DVE); LRelu; Exp -> contrib [ex*hl | ex] bf16;
  aggregation matmul per tile accumulates into per-window PSUM
  (mt_ew one-hot built on-device); window flush adds into SBUF acc (f32).
- finalize from SBUF acc: h = num/(den+1e-16) -> hT bf16 (or out rows f32).
No HBM scatter/accumulators; only per-edge gather remains indexed.
"""
import os
import sys

for _p in ("/opt/trn_rl_repo",):
    if _p not in sys.path:
        sys.path.insert(0, _p)

import numpy as np
import ml_dtypes

import concourse.bass as bass
import concourse.mybir as mybir
import concourse.tile as tile
from concourse import bacc
from concourse import library_config as libcfg
from concourse.bass_utils import run_bass_kernel_spmd

F32 = mybir.dt.float32
BF16 = mybir.dt.bfloat16
I16 = mybir.dt.int16
AX = mybir.AluOpType
AF = mybir.ActivationFunctionType
BF16NP = ml_dtypes.bfloat16


class Cfg:
    def __init__(self):
        self.N, self.E, self.C = 100000, 1600000, 8
        self.HN, self.IN_N, self.L, self.B = 64, 3, 3, 10
        self.NEG, self.EPS = 0.2, 1e-5
        self.NL = self.N // self.C          # 12500
        self.CHUNK = 25000
        self.G = self.N // self.CHUNK       # 4
        self.TE = 128
        self.BT = 32
        self.EB = self.TE * self.BT         # 4096
        self.NW = -(-self.NL // 128)        # 98
        self.NLP = self.NW * 128            # 12544


# ------------------------------------------------------------- host prep ----
def _pack_idx16(vals):
    v = np.asarray(vals, dtype=np.int16)
    assert v.size % 16 == 0
    return np.tile(v.reshape(-1, 16).T, (8, 1)).copy()


def host_prepare(cfg, inputs):
    C, G, NL, CHUNK, NW = cfg.C, cfg.G, cfg.NL, cfg.CHUNK, cfg.NW
    TE, BT, EB, HN, L = cfg.TE, cfg.BT, cfg.EB, cfg.HN, cfg.L

    ei = np.asarray(inputs["edge_index"]).astype(np.int64)
    src, dst = ei[0], ei[1]
    ea_all = np.asarray(inputs["edge_attr"], np.float32)[:, 0]
    core = dst // NL
    dstl = dst - core * NL
    gch = src // CHUNK
    win = dstl >> 7
    dof = dstl & 127

    # shared tile schedule: T_gw = max over cores of ceil(cnt/128)
    key = (core * G + gch) * NW + win
    cnt = np.bincount(key, minlength=C * G * NW).reshape(C, G, NW)
    Tgw = -(-cnt.max(axis=0) // TE)                    # [G, NW]
    tiles_g = Tgw.sum(axis=1)                          # [G]
    nbg = np.maximum(1, -(-tiles_g // BT))
    NB = int(nbg.sum())
    tile_start = np.cumsum(Tgw, axis=1) - Tgw          # [G, NW]
    block_base = np.concatenate([[0], np.cumsum(nbg)])[:G]

    # per-block window schedule (window id per tile slot, -1 = padding)
    wsched = -np.ones((NB, BT), np.int64)
    for g in range(G):
        flat = np.repeat(np.arange(NW), Tgw[g])
        b = block_base[g] + np.arange(len(flat)) // BT
        wsched[b, np.arange(len(flat)) % BT] = flat
    gob = np.repeat(np.arange(G), nbg)

    # weights
    fnw = np.asarray(inputs["fc_node_w"], np.float32)
    fnb = np.asarray(inputs["fc_node_b"], np.float32)
    few = np.asarray(inputs["fc_edge_w"], np.float32)
    feb = np.asarray(inputs["fc_edge_b"], np.float32)
    gam = np.asarray(inputs["bn_gamma"], np.float32)
    bet = np.asarray(inputs["bn_beta"], np.float32)
    fcw = np.asarray(inputs["fc_w"], np.float32)
    fcb = np.asarray(inputs["fc_b"], np.float32)
    aw = np.asarray(inputs["attn_w"], np.float32)
    ab = np.asarray(inputs["attn_b"], np.float32)
    wdst, wsrc, we = aw[:, :HN, :], aw[:, HN:2 * HN, :], aw[:, 2 * HN:, :]
    u = np.stack([few[0] @ we[l] for l in range(L)])          # [L, 64]
    adb = np.stack([ab[l] + feb @ we[l] for l in range(L)])   # [L, 64]
    x = np.asarray(inputs["x"], np.float32)

    shared = dict(
        fnw=fnw, fnb=fnb.reshape(-1, 1), gam=gam.reshape(-1, 1),
        bet=bet.reshape(-1, 1),
        fcw=np.ascontiguousarray(fcw.transpose(1, 0, 2)).astype(BF16NP),
        wsrc=np.ascontiguousarray(wsrc.transpose(1, 0, 2)).astype(BF16NP),
        wdst=np.ascontiguousarray(wdst.transpose(1, 0, 2)).astype(BF16NP),
        fcb=np.ascontiguousarray(fcb.T), adb=np.ascontiguousarray(adb.T),
        u_row=u.reshape(1, -1).astype(BF16NP),
        iota_mod=np.tile(np.arange(128, dtype=BF16NP), (128, BT)),
        identb=np.eye(128, dtype=BF16NP),
        identf=np.eye(128, dtype=np.float32),
    )

    in_maps = []
    for c in range(C):
        m = core == c
        sc, gc, wc, dfc, eac, dlc = (src[m], gch[m], win[m], dof[m],
                                     ea_all[m], dstl[m])
        order = np.lexsort((dlc, gc))
        sc, gc, wc, dfc, eac = (sc[order], gc[order], wc[order], dfc[order],
                                eac[order])
        gw = gc * NW + wc
        cnts = np.bincount(gw, minlength=G * NW)
        starts = np.cumsum(cnts) - cnts
        k = np.arange(sc.size) - starts[gw]
        tin = k >> 7
        p = k & 127
        tgi = tile_start[gc, wc] + tin
        blk = block_base[gc] + tgi // BT
        tib = tgi % BT
        slot = blk * EB + tib * TE + p

        gidx = np.zeros(NB * EB, np.int16)
        gidx[slot] = (sc - gc * CHUNK).astype(np.int16)
        mtw = np.zeros((NB, 128, EB), BF16NP)
        mtw[blk, dfc, tib * TE + p] = 1.0
        dsto = np.full((128, NB * BT), 255, BF16NP)
        dsto[p, blk * BT + tib] = dfc.astype(BF16NP)
        eab = np.zeros((NB, EB), BF16NP)
        eab[blk, tib * TE + p] = eac.astype(BF16NP)

        im = dict(shared)
        im.update(gidx=_pack_idx16(gidx), mtw=mtw, dsto=dsto, eab=eab,
                  xT=np.ascontiguousarray(x[c * NL:(c + 1) * NL].T))
        in_maps.append(im)
    return in_maps, wsched, gob, nbg


# --------------------------------------------------------------- builder ----
def split_sync_waits(nc, max_waits=1):
    for f in nc.m.functions:
        for bb in f.blocks:
            old = bb.instructions
            if not any(i.sync_info and i.sync_info.on_wait
                       and len(i.sync_info.on_wait) > max_waits for i in old):
                continue
            new = []
            for ins in old:
                si = ins.sync_info
                if si is not None and si.on_wait and len(si.on_wait) > max_waits:
                    waits = list(si.on_wait)
                    extra, keep = waits[:-max_waits], waits[-max_waits:]
                    for j, w in enumerate(extra):
                        nop = mybir.InstNoOp(name=f"{ins.name}-wc{j}", ins=[], outs=[])
                        nop.engine = ins.engine
                        nop.sync_info = mybir.SyncInfo(on_wait=[w], on_update=[])
                        new.append(nop)
                    si.on_wait = keep
                new.append(ins)
            bb.instructions = new
    return nc


def build_nc(cfg, wsched, gob, nbg):
    NL, G, CHUNK, TE, BT, EB = cfg.NL, cfg.G, cfg.CHUNK, cfg.TE, cfg.BT, cfg.EB
    HN, L, C, N, NW, NLP = cfg.HN, cfg.L, cfg.C, cfg.N, cfg.NW, cfg.NLP
    NB = int(sum(nbg))
    RG = [list(range(C))]
    # live tile count per block (live tiles form a prefix)
    nlive = [(wsched[b] >= 0).sum() for b in range(NB)]

    nc = bacc.Bacc(None, target_bir_lowering=False, num_swdge_queues=4)
    din = {}

    def ext(name, shape, dt=F32):
        din[name] = nc.dram_tensor(name, shape, dt, kind="ExternalInput")

    ext("xT", [cfg.IN_N, NL])
    ext("gidx", [128, NB * EB // 16], I16)
    ext("mtw", [NB, 128, EB], BF16)
    ext("dsto", [128, NB * BT], BF16)
    ext("eab", [NB, EB], BF16)
    ext("iota_mod", [128, BT * 128], BF16)
    ext("fnw", [cfg.IN_N, HN]); ext("fnb", [HN, 1])
    ext("gam", [HN, 1]); ext("bet", [HN, 1])
    ext("fcw", [HN, L, HN], BF16); ext("fcb", [HN, L])
    ext("wsrc", [HN, L, HN], BF16); ext("wdst", [HN, L, HN], BF16)
    ext("adb", [HN, L])
    ext("u_row", [1, L * HN], BF16)
    ext("identb", [128, 128], BF16)
    ext("identf", [128, 128])

    out = nc.dram_tensor("out", [NL, HN], F32, kind="ExternalOutput")
    local_table = nc.dram_tensor("local_table", [NL, 2 * HN], BF16)
    full_table = nc.dram_tensor("full_table", [N, 2 * HN], BF16,
                                addr_space="Shared")
    bn_loc = nc.dram_tensor("bn_loc", [HN, 2], F32)
    bn_sh = nc.dram_tensor("bn_sh", [HN, 2], F32, addr_space="Shared")

    with tile.TileContext(nc) as tc:
        with tc.tile_pool(name="persist", bufs=1) as pp:
            nc.gpsimd.load_library(libcfg.mlp)
            hT = pp.tile([HN, NLP], BF16)
            acc = pp.tile([128, NW, 2 * HN], F32)
            ad_sb = pp.tile([128, NW, HN], BF16)
            w = {}
            for nm, shp, dt in (
                ("fnw", [cfg.IN_N, HN], F32), ("fnb", [HN, 1], F32),
                ("gam", [HN, 1], F32), ("bet", [HN, 1], F32),
                ("fcw", [HN, L, HN], BF16), ("fcb", [HN, L], F32),
                ("wsrc", [HN, L, HN], BF16), ("wdst", [HN, L, HN], BF16),
                ("adb", [HN, L], F32),
                ("u_row", [1, L * HN], BF16),
                ("identb", [128, 128], BF16), ("identf", [128, 128], F32),
                ("dsto", [128, NB * BT], BF16),
                ("iota_mod", [128, BT * 128], BF16),
            ):
                w[nm] = pp.tile(shp, dt, tag=nm, name=nm)
                nc.sync.dma_start(out=w[nm][:], in_=din[nm][:])
            nc.vector.memset(ad_sb[:], 0.0)

            # ---------------- input + BN ----------------
            with tc.tile_pool(name="bn", bufs=2) as bp, \
                 tc.tile_pool(name="bnc", bufs=3) as bq, \
                 tc.tile_pool(name="bnp", bufs=2, space="PSUM") as bpp:
                st = bp.tile([HN, 2], F32, tag="st")
                nc.vector.memset(st[:], 0.0)
                for ci in range(-(-NL // 512)):
                    c0 = ci * 512
                    n = min(512, NL - c0)
                    xt = bq.tile([cfg.IN_N, 512], F32, tag="xt")
                    nc.sync.dma_start(out=xt[:, :n], in_=din["xT"][:, c0:c0 + n])
                    ps = bpp.tile([HN, 512], F32, tag="ps")
                    nc.tensor.matmul(out=ps[:, :n], lhsT=w["fnw"][:],
                                     rhs=xt[:, :n], start=True, stop=True)
                    nc.vector.tensor_scalar_add(hT[:, c0:c0 + n], ps[:, :n],
                                                w["fnb"][:])
                    stc = bq.tile([HN, 2], F32, tag="stc")
                    nc.vector.reduce_sum(stc[:, 0:1], hT[:, c0:c0 + n],
                                         axis=mybir.AxisListType.X)
                    sq = bq.tile([HN, 512], F32, tag="sq")
                    nc.vector.scalar_tensor_tensor(
                        out=sq[:, :n], in0=hT[:, c0:c0 + n], scalar=1.0,
                        in1=hT[:, c0:c0 + n], op0=AX.mult, op1=AX.mult,
                        accum_out=stc[:, 1:2])
                    nc.vector.tensor_add(st[:], st[:], stc[:])
                nc.sync.dma_start(out=bn_loc[:], in_=st[:])
                nc.gpsimd.collective_compute("AllReduce", AX.add,
                                             replica_groups=RG,
                                             ins=[bn_loc[:]], outs=[bn_sh[:]])
                sg = bp.tile([HN, 2], F32, tag="sg")
                nc.sync.dma_start(out=sg[:], in_=bn_sh[:])
                mean = bp.tile([HN, 1], F32, tag="mean")
                var = bp.tile([HN, 1], F32, tag="var")
                nc.vector.tensor_scalar_mul(mean[:], sg[:, 0:1], 1.0 / N)
                nc.vector.tensor_scalar_mul(var[:], sg[:, 1:2], 1.0 / N)
                msq = bp.tile([HN, 1], F32, tag="msq")
                nc.vector.tensor_mul(msq[:], mean[:], mean[:])
                nc.vector.tensor_sub(var[:], var[:], msq[:])
                nc.vector.tensor_scalar_add(var[:], var[:], cfg.EPS)
                rs = bp.tile([HN, 1], F32, tag="rs")
                nc.scalar.activation(out=rs[:], in_=var[:], func=AF.Sqrt)
                nc.vector.reciprocal(rs[:], rs[:])
                scale = bp.tile([HN, 1], F32, tag="scale")
                nc.vector.tensor_mul(scale[:], rs[:], w["gam"][:])
                nbias = bp.tile([HN, 1], F32, tag="nbias")
                nc.vector.tensor_mul(nbias[:], mean[:], scale[:])
                nc.vector.scalar_tensor_tensor(out=nbias[:], in0=nbias[:],
                                               scalar=-1.0, in1=w["bet"][:],
                                               op0=AX.mult, op1=AX.add)
                nc.vector.tensor_scalar(out=hT[:, :NL], in0=hT[:, :NL],
                                        scalar1=scale[:], scalar2=nbias[:],
                                        op0=AX.mult, op1=AX.add)

            # ---------------- layers (interleaved emission) ----------------
            # last-touch block for each window; finalize/table emitted there
            lt = {}
            for b in range(NB):
                for t in range(int(nlive[b])):
                    lt[int(wsched[b, t])] = b
            fin_after = {b: [] for b in range(NB)}
            for wv in range(NW):
                fin_after[lt.get(wv, NB - 1)].append(wv)

            with tc.tile_pool(name="tb", bufs=3) as tp, \
                 tc.tile_pool(name="tq", bufs=1, space="PSUM") as tqq, \
                 tc.tile_pool(name="tp2", bufs=1, space="PSUM") as tpp, \
                 tc.tile_pool(name="ep", bufs=2) as ep, \
                 tc.tile_pool(name="pp", bufs=2, space="PSUM") as epp, \
                 tc.tile_pool(name="wp", bufs=3, space="PSUM") as wpp:

                def table_build(l, wv):
                    # hT cols [wv*128, +n) -> local_table rows + ad_sb window
                    c0 = wv * 128
                    n = min(128, NL - c0)
                    psh = tpp.tile([HN, 128], F32, tag="psh")
                    nc.tensor.matmul(out=psh[:, :n], lhsT=w["fcw"][:, l, :],
                                     rhs=hT[:, c0:c0 + n], start=True, stop=True)
                    hl = tp.tile([HN, 128], BF16, tag="hl")
                    nc.vector.tensor_scalar_add(hl[:, :n], psh[:, :n],
                                                w["fcb"][:, l:l + 1])
                    pss = tpp.tile([HN, 128], F32, tag="psh")
                    nc.tensor.matmul(out=pss[:, :n], lhsT=w["wsrc"][:, l, :],
                                     rhs=hl[:, :n], start=True, stop=True)
                    asb = tp.tile([HN, 128], BF16, tag="asb")
                    nc.scalar.copy(out=asb[:, :n], in_=pss[:, :n])
                    psd = tpp.tile([HN, 128], F32, tag="psh")
                    nc.tensor.matmul(out=psd[:, :n], lhsT=w["wdst"][:, l, :],
                                     rhs=hl[:, :n], start=True, stop=True)
                    adt = tp.tile([HN, 128], BF16, tag="adt")
                    nc.vector.tensor_scalar_add(adt[:, :n], psd[:, :n],
                                                w["adb"][:, l:l + 1])
                    rows = tp.tile([128, 2 * HN], BF16, tag="rows")
                    pt = tpp.tile([128, HN], BF16, tag="pt")
                    nc.tensor.transpose(out=pt[:n, :], in_=asb[:, :n],
                                        identity=w["identb"][:HN, :HN])
                    nc.scalar.copy(out=rows[:n, 0:HN], in_=pt[:n, :])
                    pt2 = tpp.tile([128, HN], BF16, tag="pt")
                    nc.tensor.transpose(out=pt2[:n, :], in_=hl[:, :n],
                                        identity=w["identb"][:HN, :HN])
                    nc.scalar.copy(out=rows[:n, HN:2 * HN], in_=pt2[:n, :])
                    nc.sync.dma_start(out=local_table[c0:c0 + n, :],
                                      in_=rows[:n, :])
                    pt3 = tpp.tile([128, HN], BF16, tag="pt")
                    nc.tensor.transpose(out=pt3[:n, :], in_=adt[:, :n],
                                        identity=w["identb"][:HN, :HN])
                    nc.vector.tensor_copy(ad_sb[:n, wv, :], pt3[:n, :])

                def finalize_hT(wv):
                    ptr = tqq.tile([128, 128], F32, tag="ptr")
                    nc.tensor.transpose(out=ptr[:], in_=acc[:, wv, :],
                                        identity=w["identf"][:])
                    den = tp.tile([HN, 128], F32, tag="den")
                    nc.vector.tensor_scalar_add(den[:], ptr[HN:2 * HN, :], 1e-16)
                    nc.vector.reciprocal(den[:], den[:])
                    nc.vector.tensor_mul(hT[:, wv * 128:(wv + 1) * 128],
                                         ptr[0:HN, :], den[:])

                def output_rows(wv):
                    n = min(128, NL - wv * 128)
                    den = tp.tile([128, HN], F32, tag="den2")
                    nc.vector.tensor_scalar_add(
                        den[:n, :], acc[:n, wv, HN:2 * HN], 1e-16)
                    nc.vector.reciprocal(den[:n, :], den[:n, :])
                    orow = tp.tile([128, HN], F32, tag="orow")
                    nc.vector.tensor_mul(orow[:n, :], acc[:n, wv, 0:HN],
                                         den[:n, :])
                    nc.sync.dma_start(out=out[wv * 128:wv * 128 + n, :],
                                      in_=orow[:n, :])

                def emit_block(l, b):
                    g = int(gob[b])
                    nl_b = int(nlive[b])
                    gix = ep.tile([128, EB // 16], I16, tag="gix")
                    nc.sync.dma_start(
                        out=gix[:],
                        in_=din["gidx"][:, b * (EB // 16):(b + 1) * (EB // 16)])
                    mtw_t = ep.tile([128, EB], BF16, tag="mtw_t")
                    nc.sync.dma_start(out=mtw_t[:], in_=din["mtw"][b, :, :])
                    eat = ep.tile([1, EB], BF16, tag="eat", bufs=1)
                    nc.sync.dma_start(out=eat[:], in_=din["eab"][b:b + 1, :])
                    mt_ew = ep.tile([128, BT, 128], BF16, tag="mt_ew")
                    nc.vector.tensor_tensor(
                        out=mt_ew[:],
                        in0=w["dsto"][:, b * BT:(b + 1) * BT].rearrange(
                            "p (t o) -> p t o", o=1).to_broadcast([128, BT, 128]),
                        in1=w["iota_mod"][:].rearrange("p (t s) -> p t s", s=128),
                        op=AX.is_equal)
                    srcr = ep.tile([128, BT, 2 * HN], BF16, tag="srcr")
                    for hh in range(4):
                        live = min(EB // 4, max(0, nl_b * TE - hh * (EB // 4)))
                        if live == 0:
                            continue
                        nc.gpsimd.dma_gather(
                            out_ap=srcr[:, hh * (BT // 4):(hh + 1) * (BT // 4), :],
                            in_ap=full_table[g * CHUNK:(g + 1) * CHUNK, :],
                            idxs_ap=gix[:, hh * (EB // 64):(hh + 1) * (EB // 64)],
                            num_idxs=EB // 4, num_idxs_reg=live,
                            elem_size=2 * HN,
                            single_packet=True, queue_num=(b + hh) % 4)
                    alpha = ep.tile([128, BT, HN], F32, tag="alpha")
                    contrib = ep.tile([128, BT, 2 * HN], BF16, tag="contrib")
                    for grp in range(-(-nl_b // 8)):
                        t0 = grp * 8
                        gn = min(8, nl_b - t0)
                        psx = epp.tile([128, 8 * HN], F32, tag="psx")
                        for t2 in range(gn):
                            t = t0 + t2
                            nc.tensor.matmul(
                                out=psx[:, t2 * HN:(t2 + 1) * HN],
                                lhsT=mtw_t[:, t * TE:(t + 1) * TE],
                                rhs=ad_sb[:, int(wsched[b, t]), :],
                                start=True, stop=False)
                            nc.tensor.matmul(
                                out=psx[:, t2 * HN:(t2 + 1) * HN],
                                lhsT=eat[0:1, t * TE:(t + 1) * TE],
                                rhs=w["u_row"][:, l * HN:(l + 1) * HN],
                                start=False, stop=True)
                        nc.vector.tensor_tensor(
                            out=alpha[:, t0:t0 + gn, :],
                            in0=psx[:, :gn * HN].rearrange(
                                "p (t h) -> p t h", h=HN),
                            in1=srcr[:, t0:t0 + gn, 0:HN],
                            op=AX.add)
                    extmp = ep.tile([128, BT, HN], BF16, tag="extmp")
                    nc.scalar.activation(out=contrib[:, :nl_b, HN:2 * HN],
                                         in_=alpha[:, :nl_b, :], func=AF.Exp)
                    nc.scalar.activation(out=extmp[:, :nl_b, :],
                                         in_=alpha[:, :nl_b, :], func=AF.Exp,
                                         scale=cfg.NEG)
                    nc.vector.tensor_tensor(
                        out=contrib[:, :nl_b, HN:2 * HN],
                        in0=contrib[:, :nl_b, HN:2 * HN],
                        in1=extmp[:, :nl_b, :], op=AX.max)
                    nc.vector.tensor_mul(contrib[:, :nl_b, 0:HN],
                                         contrib[:, :nl_b, HN:2 * HN],
                                         srcr[:, :nl_b, HN:2 * HN])
                    t = 0
                    while t < nl_b:
                        wv = int(wsched[b, t])
                        t1 = t
                        while t1 + 1 < nl_b and wsched[b, t1 + 1] == wv:
                            t1 += 1
                        wps = wpp.tile([128, 2 * HN], F32, tag="wps")
                        for tt in range(t, t1 + 1):
                            nc.tensor.matmul(
                                out=wps[:], lhsT=mt_ew[:, tt, :],
                                rhs=contrib[:, tt, :],
                                start=(tt == t), stop=(tt == t1))
                        nc.vector.tensor_add(acc[:, wv, :], acc[:, wv, :],
                                             wps[:])
                        t = t1 + 1

                # layer 0 table from BN hT + acc init
                for wv in range(NW):
                    table_build(0, wv)
                    nc.vector.memset(acc[:, wv, :], 0.0)
                nc.gpsimd.collective_compute("AllGather", AX.bypass,
                                             replica_groups=RG,
                                             ins=[local_table[:]],
                                             outs=[full_table[:]])
                for l in range(L):
                    for b in range(NB):
                        emit_block(l, b)
                        for wv in fin_after[b]:
                            if l < L - 1:
                                finalize_hT(wv)
                                table_build(l + 1, wv)
                                nc.vector.memset(acc[:, wv, :], 0.0)
                            else:
                                output_rows(wv)
                    if l < L - 1:
                        nc.gpsimd.collective_compute(
                            "AllGather", AX.bypass, replica_groups=RG,
                            ins=[local_table[:]], outs=[full_table[:]])
    return nc


# --------------------------------------------------------------- entry ------
def kernel(**inputs):
    cfg = Cfg()
    in_maps, wsched, gob, nbg = host_prepare(cfg, inputs)
    nc = build_nc(cfg, wsched, gob, nbg)
    nc.compile()
    split_sync_waits(nc)
    res = run_bass_kernel_spmd(nc, in_maps, core_ids=list(range(cfg.C)),
                               trace=bool(int(os.environ.get("GNN_TRACE", "0"))))
    if res.exec_time_ns is not None:
        print(f"HW exec time: {res.exec_time_ns} ns")
    if res.instructions_and_trace is not None:
        print(f"trace: {res.instructions_and_trace[1]}")
    h = np.concatenate([np.asarray(res.results[c]["out"])
                        for c in range(cfg.C)], axis=0)
    return h.reshape(cfg.B, -1, cfg.HN)
